# revision 10
# baseline (speedup 1.0000x reference)
"""RWKV-4 block (TimeMix + ChannelMix) Trainium2 Bass kernel.

Full inputs in, full outputs out. Sharding: data-parallel over batch B=8
across the 8 NeuronCores (one batch element per core, no collectives).

Per-core layout strategy:
  - LayerNorm + stats in token-major [128 tokens, C] (bn_stats along free dim),
    then PE-transpose to feature-major [128 chan, T] for everything else.
  - token_shift is a free-dim offset view of a front-padded feature-major tile.
  - k/v/r and FFN matmuls in bf16 (fp32 PSUM accumulation).
  - The WKV recurrence A_t = lam*A_{t-1} + e^{k_t} v_t  (and same for B with
    e^{k_t}) runs as a native DVE tensor_tensor_scan per 128-channel block,
    fp32, chained across T-chunks via the `initial` operand.
    y_t = (A_{t-1} + e^u e^{k_t} v_t) / (B_{t-1} + e^u e^{k_t}) -- division
    via ACT Ln+Exp (1/x = exp(-ln x), denominator is strictly positive).
"""

import os
import sys

import numpy as np

if "/opt/trn_rl_repo" not in sys.path:
    sys.path.insert(0, "/opt/trn_rl_repo")

import ml_dtypes

B, T, C, FFN = 8, 2048, 1024, 4096
NCORES = 8
CB = C // 128      # 8 channel blocks
FB = FFN // 128    # 32 ffn blocks
TCA = 512          # TimeMix chunk (tokens)
NCHA = T // TCA    # 4
TCB = 256          # FFN chunk (tokens)
NCHB = T // TCB    # 8
LN_EPS = 1e-5

BF16 = None  # set lazily (mybir.dt.bfloat16)

_CACHE = {}


def _build_program():
    import concourse.bass as bass  # noqa: F401
    from concourse import bacc
    import concourse.tile as tile
    import concourse.mybir as mybir
    from concourse.masks import make_identity
    from contextlib import ExitStack

    f32 = mybir.dt.float32
    bf16 = mybir.dt.bfloat16
    AF = mybir.ActivationFunctionType
    OP = mybir.AluOpType

    nc = bacc.Bacc("TRN2", target_bir_lowering=False, debug=False,
                   enable_asserts=False)

    # ---------------- DRAM I/O ----------------
    x_d = nc.dram_tensor("x", [T, C], f32, kind="ExternalInput").ap()
    y_d = nc.dram_tensor("y", [T, C], f32, kind="ExternalOutput").ap()

    wk_d = nc.dram_tensor("wkT", [C, C], bf16, kind="ExternalInput").ap()
    wv_d = nc.dram_tensor("wvT", [C, C], bf16, kind="ExternalInput").ap()
    wr_d = nc.dram_tensor("wrT", [C, C], bf16, kind="ExternalInput").ap()
    wo_d = nc.dram_tensor("woT", [C, C], bf16, kind="ExternalInput").ap()
    wkf_d = nc.dram_tensor("wkfT", [C, FFN], bf16, kind="ExternalInput").ap()
    wvf_d = nc.dram_tensor("wvfT", [FFN, C], bf16, kind="ExternalInput").ap()
    wrf_d = nc.dram_tensor("wrfT", [C, C], bf16, kind="ExternalInput").ap()

    PARAMS = ["mk", "mv", "mr", "mfk", "mfr", "lam", "eu", "g1", "b1", "g2", "b2"]
    par_d = {p: nc.dram_tensor(p, [C], f32, kind="ExternalInput").ap()
             for p in PARAMS}

    with tile.TileContext(nc) as tc, ExitStack() as glob:
        const = glob.enter_context(tc.tile_pool(name="const", bufs=1))
        psT = glob.enter_context(tc.tile_pool(name="psT", bufs=2, space="PSUM"))
        dram = glob.enter_context(tc.tile_pool(name="dram", bufs=1, space="DRAM"))

        # per-channel params as [128, CB] tiles: column cb = channels cb*128..+128
        par = {}
        for p in PARAMS:
            pt = const.tile([128, CB], f32, name=f"p_{p}")
            nc.sync.dma_start(out=pt, in_=par_d[p].rearrange("(a p) -> p a", p=128))
            par[p] = pt

        ident = const.tile([128, 128], f32)
        make_identity(nc, ident)

        eps_t = const.tile([128, 1], f32)
        nc.vector.memset(eps_t, LN_EPS)
        one_t = const.tile([128, 1], f32)
        nc.vector.memset(one_t, 1.0)

        Acar = const.tile([128, CB], f32)
        nc.vector.memset(Acar, 0.0)
        Bcar = const.tile([128, CB], f32)
        nc.vector.memset(Bcar, 0.0)
        xlast = const.tile([128, CB], bf16)
        xlast2 = const.tile([128, CB], bf16)

        x2_dram = dram.tile([T, C], f32)

        # ---- helper: LN stats + apply for one [128, C] token tile ----
        def ln_apply(xt, statsp, xnp):
            st6 = statsp.tile([128, 2, 6], f32, name="st6", tag="st6")
            nc.vector.bn_stats(out=st6[:, 0, :], in_=xt[:, 0:512])
            nc.vector.bn_stats(out=st6[:, 1, :], in_=xt[:, 512:1024])
            mv_t = statsp.tile([128, 2], f32, name="mv_t", tag="mv")
            nc.vector.bn_aggr(out=mv_t, in_=st6)
            # rstd = exp(-0.5*ln(var+eps)) — keeps ACT inside the ln/exp
            # table set (Sqrt lives in a different set -> ~2.7us reload).
            ldv = statsp.tile([128, 1], f32, name="ldv", tag="ldv")
            nc.scalar.activation(out=ldv, in_=mv_t[:, 1:2], func=AF.Ln,
                                 bias=eps_t)
            rstd = statsp.tile([128, 1], f32, name="rstd", tag="rstd")
            nc.scalar.activation(out=rstd, in_=ldv, func=AF.Exp, scale=-0.5)
            xnh = xnp.tile([128, C], f32, name="xnh", tag="xnh")
            nc.vector.tensor_scalar(out=xnh, in0=xt, scalar1=mv_t[:, 0:1],
                                    scalar2=rstd, op0=OP.subtract, op1=OP.mult)
            return xnh

        # ================= Phase A: TimeMix =================
        with ExitStack() as ph:
            wp = ph.enter_context(tc.tile_pool(name="wA", bufs=1))
            wk_sb = wp.tile([128, CB, C], bf16)
            wv_sb = wp.tile([128, CB, C], bf16)
            wr_sb = wp.tile([128, CB, C], bf16)
            wo_sb = wp.tile([128, CB, C], bf16)
            for cb in range(CB):
                nc.sync.dma_start(out=wk_sb[:, cb, :], in_=wk_d[cb * 128:(cb + 1) * 128, :])
                nc.sync.dma_start(out=wv_sb[:, cb, :], in_=wv_d[cb * 128:(cb + 1) * 128, :])
                nc.sync.dma_start(out=wr_sb[:, cb, :], in_=wr_d[cb * 128:(cb + 1) * 128, :])
                nc.sync.dma_start(out=wo_sb[:, cb, :], in_=wo_d[cb * 128:(cb + 1) * 128, :])

            xtp = ph.enter_context(tc.tile_pool(name="xtp", bufs=5))
            statsp = ph.enter_context(tc.tile_pool(name="statsA", bufs=4))
            xnp = ph.enter_context(tc.tile_pool(name="xnpA", bufs=2))
            xnTp = ph.enter_context(tc.tile_pool(name="xnTA", bufs=1))
            mixp = ph.enter_context(tc.tile_pool(name="mixA", bufs=1))
            wkvp = ph.enter_context(tc.tile_pool(name="wkv", bufs=2))
            x2p = ph.enter_context(tc.tile_pool(name="x2p", bufs=2))
            psA = ph.enter_context(tc.tile_pool(name="psA", bufs=4, space="PSUM"))
            psO = ph.enter_context(tc.tile_pool(name="psO", bufs=2, space="PSUM"))

            NTT = TCA // 128  # 4 token tiles per chunk

            for ic in range(NCHA):
                t0 = ic * TCA
                xts = []
                for tt in range(NTT):
                    xt = xtp.tile([128, C], f32, name="xt", tag="xt")
                    nc.sync.dma_start(
                        out=xt, in_=x_d[t0 + tt * 128: t0 + (tt + 1) * 128, :])
                    xts.append(xt)

                # feature-major, front-padded by one column for token_shift
                xnT = xnTp.tile([128, CB, TCA + 1], bf16, name="xnT", tag="xnT")
                if ic == 0:
                    nc.gpsimd.memset(xnT[:, :, 0], 0.0)
                else:
                    nc.gpsimd.tensor_copy(out=xnT[:, :, 0], in_=xlast)

                for tt in range(NTT):
                    xnh = ln_apply(xts[tt], statsp, xnp)
                    for cb in range(CB):
                        pt = psT.tile([128, 128], f32, name="pt", tag="pt")
                        nc.tensor.transpose(pt, xnh[:, cb * 128:(cb + 1) * 128], ident)
                        nc.scalar.activation(
                            out=xnT[:, cb, 1 + tt * 128: 1 + (tt + 1) * 128],
                            in_=pt, func=AF.Identity,
                            bias=par["b1"][:, cb:cb + 1],
                            scale=par["g1"][:, cb:cb + 1])
                nc.gpsimd.tensor_copy(out=xlast, in_=xnT[:, :, TCA])

                xk_t = mixp.tile([128, CB, TCA], bf16, name="xk_t", tag="xk")
                xv_t = mixp.tile([128, CB, TCA], bf16, name="xv_t", tag="xv")
                xr_t = mixp.tile([128, CB, TCA], bf16, name="xr_t", tag="xr")
                for cb in range(CB):
                    xn_v = xnT[:, cb, 1:TCA + 1]   # xn[t]
                    xx_v = xnT[:, cb, 0:TCA]       # xn[t-1]
                    dd = mixp.tile([128, TCA], bf16, name="dd", tag="dd", bufs=2)
                    nc.vector.tensor_sub(dd, xn_v, xx_v)
                    nc.vector.scalar_tensor_tensor(
                        out=xk_t[:, cb, :], in0=dd, scalar=par["mk"][:, cb:cb + 1],
                        in1=xx_v, op0=OP.mult, op1=OP.add)
                    nc.vector.scalar_tensor_tensor(
                        out=xv_t[:, cb, :], in0=dd, scalar=par["mv"][:, cb:cb + 1],
                        in1=xx_v, op0=OP.mult, op1=OP.add)
                    nc.vector.scalar_tensor_tensor(
                        out=xr_t[:, cb, :], in0=dd, scalar=par["mr"][:, cb:cb + 1],
                        in1=xx_v, op0=OP.mult, op1=OP.add)

                rw_t = wkvp.tile([128, CB, TCA], bf16, name="rw_t", tag="rw")
                for db in range(CB):
                    dsl = slice(db * 128, (db + 1) * 128)
                    psk = psA.tile([128, TCA], f32, name="psk", tag="mmA")
                    for cb in range(CB):
                        nc.tensor.matmul(psk, wk_sb[:, cb, dsl], xk_t[:, cb, :],
                                         start=(cb == 0), stop=(cb == CB - 1))
                    ek = wkvp.tile([128, TCA], f32, name="ek", tag="ek")
                    nc.scalar.activation(out=ek, in_=psk, func=AF.Exp)

                    psv = psA.tile([128, TCA], f32, name="psv", tag="mmA")
                    for cb in range(CB):
                        nc.tensor.matmul(psv, wv_sb[:, cb, dsl], xv_t[:, cb, :],
                                         start=(cb == 0), stop=(cb == CB - 1))
                    ekv = wkvp.tile([128, TCA], f32, name="ekv", tag="ekv")
                    nc.vector.tensor_mul(ekv, ek, psv)

                    psr = psA.tile([128, TCA], f32, name="psr", tag="mmA")
                    for cb in range(CB):
                        nc.tensor.matmul(psr, wr_sb[:, cb, dsl], xr_t[:, cb, :],
                                         start=(cb == 0), stop=(cb == CB - 1))
                    # sigmoid(r)*num/den == num / (den*(1+exp(-r))) — avoids
                    # the Sigmoid table set; everything stays in ln/exp.
                    er = wkvp.tile([128, TCA], f32, name="er", tag="sr")
                    nc.scalar.activation(out=er, in_=psr, func=AF.Exp, scale=-1.0)

                    lam_s = par["lam"][:, db:db + 1]
                    Apad = wkvp.tile([128, TCA + 1], f32, name="Apad", tag="Apad")
                    nc.gpsimd.tensor_copy(out=Apad[:, 0:1], in_=Acar[:, db:db + 1])
                    nc.vector.tensor_tensor_scan(
                        out=Apad[:, 1:TCA + 1], data0=lam_s.broadcast_to([128, TCA]),
                        data1=ekv, initial=Acar[:, db:db + 1],
                        op0=OP.mult, op1=OP.add)
                    nc.gpsimd.tensor_copy(out=Acar[:, db:db + 1],
                                          in_=Apad[:, TCA:TCA + 1])

                    Bpad = wkvp.tile([128, TCA + 1], f32, name="Bpad", tag="Bpad")
                    nc.gpsimd.tensor_copy(out=Bpad[:, 0:1], in_=Bcar[:, db:db + 1])
                    nc.vector.tensor_tensor_scan(
                        out=Bpad[:, 1:TCA + 1], data0=lam_s.broadcast_to([128, TCA]),
                        data1=ek, initial=Bcar[:, db:db + 1],
                        op0=OP.mult, op1=OP.add)
                    nc.gpsimd.tensor_copy(out=Bcar[:, db:db + 1],
                                          in_=Bpad[:, TCA:TCA + 1])

                    eu_s = par["eu"][:, db:db + 1]
                    num = wkvp.tile([128, TCA], f32, name="num", tag="num")
                    nc.vector.scalar_tensor_tensor(
                        out=num, in0=ekv, scalar=eu_s, in1=Apad[:, 0:TCA],
                        op0=OP.mult, op1=OP.add)
                    den = wkvp.tile([128, TCA], f32, name="den", tag="den")
                    nc.vector.scalar_tensor_tensor(
                        out=den, in0=ek, scalar=eu_s, in1=Bpad[:, 0:TCA],
                        op0=OP.mult, op1=OP.add)
                    den2 = wkvp.tile([128, TCA], f32, name="den2", tag="den2")
                    nc.vector.scalar_tensor_tensor(
                        out=den2, in0=er, scalar=1.0, in1=den,
                        op0=OP.add, op1=OP.mult)
                    ldn = wkvp.tile([128, TCA], f32, name="ldn", tag="ldn")
                    nc.scalar.activation(out=ldn, in_=den2, func=AF.Ln)
                    inv = wkvp.tile([128, TCA], f32, name="inv", tag="inv")
                    nc.scalar.activation(out=inv, in_=ldn, func=AF.Exp, scale=-1.0)
                    nc.vector.tensor_mul(rw_t[:, db, :], num, inv)

                # Wo back to token-major + residual
                for tt in range(NTT):
                    for chh in range(2):
                        pso = psO.tile([128, 512], f32, name="pso", tag="mmO")
                        for db in range(CB):
                            nc.tensor.matmul(
                                pso,
                                rw_t[:, db, tt * 128:(tt + 1) * 128],
                                wo_sb[:, db, chh * 512:(chh + 1) * 512],
                                start=(db == 0), stop=(db == CB - 1))
                        x2sb = x2p.tile([128, 512], f32, name="x2sb", tag="x2")
                        nc.vector.tensor_add(
                            x2sb, xts[tt][:, chh * 512:(chh + 1) * 512], pso)
                        nc.sync.dma_start(
                            out=x2_dram[t0 + tt * 128: t0 + (tt + 1) * 128,
                                        chh * 512:(chh + 1) * 512],
                            in_=x2sb)

        # ================= Phase B: ChannelMix =================
        with ExitStack() as ph:
            wp = ph.enter_context(tc.tile_pool(name="wB", bufs=1))
            wkf_sb = wp.tile([128, CB, FFN], bf16)
            for cb in range(CB):
                nc.sync.dma_start(out=wkf_sb[:, cb, :],
                                  in_=wkf_d[cb * 128:(cb + 1) * 128, :])
            wvf_sb = wp.tile([128, FB, C], bf16)
            for fb in range(FB):
                nc.sync.dma_start(out=wvf_sb[:, fb, :],
                                  in_=wvf_d[fb * 128:(fb + 1) * 128, :])
            wrf_sb = wp.tile([128, CB, C], bf16)
            for cb in range(CB):
                nc.sync.dma_start(out=wrf_sb[:, cb, :],
                                  in_=wrf_d[cb * 128:(cb + 1) * 128, :])

            x2tp = ph.enter_context(tc.tile_pool(name="x2tp", bufs=3))
            statsp = ph.enter_context(tc.tile_pool(name="statsB", bufs=4))
            xnp = ph.enter_context(tc.tile_pool(name="xnpB", bufs=2))
            xnTp = ph.enter_context(tc.tile_pool(name="xnTB", bufs=1))
            mixp = ph.enter_context(tc.tile_pool(name="mixB", bufs=1))
            ffp = ph.enter_context(tc.tile_pool(name="ffp", bufs=2))
            psKV = ph.enter_context(tc.tile_pool(name="psKV", bufs=1, space="PSUM"))
            psSm = ph.enter_context(tc.tile_pool(name="psSm", bufs=2, space="PSUM"))

            NTT = TCB // 128  # 2

            for jc in range(NCHB):
                t0 = jc * TCB
                x2ts = []
                for tt in range(NTT):
                    x2t = x2tp.tile([128, C], f32, name="x2t", tag="x2t")
                    nc.sync.dma_start(
                        out=x2t, in_=x2_dram[t0 + tt * 128: t0 + (tt + 1) * 128, :])
                    x2ts.append(x2t)

                xnT = xnTp.tile([128, CB, TCB + 1], bf16, name="xnT2", tag="xnT2")
                if jc == 0:
                    nc.gpsimd.memset(xnT[:, :, 0], 0.0)
                else:
                    nc.gpsimd.tensor_copy(out=xnT[:, :, 0], in_=xlast2)

                for tt in range(NTT):
                    xnh = ln_apply(x2ts[tt], statsp, xnp)
                    for cb in range(CB):
                        pt = psT.tile([128, 128], f32, name="pt2", tag="pt")
                        nc.tensor.transpose(pt, xnh[:, cb * 128:(cb + 1) * 128], ident)
                        nc.scalar.activation(
                            out=xnT[:, cb, 1 + tt * 128: 1 + (tt + 1) * 128],
                            in_=pt, func=AF.Identity,
                            bias=par["b2"][:, cb:cb + 1],
                            scale=par["g2"][:, cb:cb + 1])
                nc.gpsimd.tensor_copy(out=xlast2, in_=xnT[:, :, TCB])

                xk_t = mixp.tile([128, CB, TCB], bf16, name="xk2_t", tag="xk2")
                xr_t = mixp.tile([128, CB, TCB], bf16, name="xr2_t", tag="xr2")
                for cb in range(CB):
                    xn_v = xnT[:, cb, 1:TCB + 1]
                    xx_v = xnT[:, cb, 0:TCB]
                    dd = mixp.tile([128, TCB], bf16, name="dd2", tag="dd2", bufs=2)
                    nc.vector.tensor_sub(dd, xn_v, xx_v)
                    nc.vector.scalar_tensor_tensor(
                        out=xk_t[:, cb, :], in0=dd, scalar=par["mfk"][:, cb:cb + 1],
                        in1=xx_v, op0=OP.mult, op1=OP.add)
                    nc.vector.scalar_tensor_tensor(
                        out=xr_t[:, cb, :], in0=dd, scalar=par["mfr"][:, cb:cb + 1],
                        in1=xx_v, op0=OP.mult, op1=OP.add)

                kvps = [[psKV.tile([128, 512], f32, name=f"kv{tt}{chh}",
                                   tag=f"kv{tt}{chh}")
                         for chh in range(2)] for tt in range(NTT)]

                for fb in range(FB):
                    pskf = psSm.tile([128, TCB], f32, name="pskf", tag="sm")
                    for cb in range(CB):
                        nc.tensor.matmul(
                            pskf, wkf_sb[:, cb, fb * 128:(fb + 1) * 128],
                            xk_t[:, cb, :], start=(cb == 0), stop=(cb == CB - 1))
                    kfb = ffp.tile([128, TCB], bf16, name="kfb", tag="kf", bufs=3)
                    nc.scalar.activation(out=kfb, in_=pskf, func=AF.Relu)
                    kfs = ffp.tile([128, TCB], bf16, name="kfs", tag="kfs", bufs=3)
                    nc.vector.tensor_mul(kfs, kfb, kfb)
                    for tt in range(NTT):
                        for chh in range(2):
                            nc.tensor.matmul(
                                kvps[tt][chh],
                                kfs[:, tt * 128:(tt + 1) * 128],
                                wvf_sb[:, fb, chh * 512:(chh + 1) * 512],
                                start=(fb == 0), stop=(fb == FB - 1))

                for tt in range(NTT):
                    for chh in range(2):
                        psr2 = psSm.tile([128, 512], f32, name="psr2", tag="sm")
                        for cb in range(CB):
                            nc.tensor.matmul(
                                psr2,
                                xr_t[:, cb, tt * 128:(tt + 1) * 128],
                                wrf_sb[:, cb, chh * 512:(chh + 1) * 512],
                                start=(cb == 0), stop=(cb == CB - 1))
                        # sigmoid via exp/ln only (same ACT table set):
                        # sig = exp(-ln(1+exp(-r)))
                        er2 = ffp.tile([128, 512], f32, name="er2", tag="sr2")
                        nc.scalar.activation(out=er2, in_=psr2, func=AF.Exp,
                                             scale=-1.0)
                        ln2t = ffp.tile([128, 512], f32, name="ln2t", tag="ln2t")
                        nc.scalar.activation(out=ln2t, in_=er2, func=AF.Ln,
                                             bias=one_t)
                        sg2 = ffp.tile([128, 512], f32, name="sg2", tag="sg2")
                        nc.scalar.activation(out=sg2, in_=ln2t, func=AF.Exp,
                                             scale=-1.0)
                        tmp = ffp.tile([128, 512], f32, name="tmp", tag="tmp", bufs=1)
                        nc.vector.tensor_mul(tmp, sg2, kvps[tt][chh])
                        x3 = ffp.tile([128, 512], f32, name="x3", tag="x3")
                        nc.vector.tensor_add(
                            x3, tmp, x2ts[tt][:, chh * 512:(chh + 1) * 512])
                        nc.sync.dma_start(
                            out=y_d[t0 + tt * 128: t0 + (tt + 1) * 128,
                                    chh * 512:(chh + 1) * 512],
                            in_=x3)

    nc.compile()
    return nc


def get_program():
    if "nc" not in _CACHE:
        _CACHE["nc"] = _build_program()
    return _CACHE["nc"]


def host_inputs(inputs):
    """Host-side precompute: transposed bf16 weights + derived per-channel vecs.
    Returns the shared (per-core-identical) part of the in_map."""
    bf = ml_dtypes.bfloat16
    f32 = np.float32

    def v(name):
        return np.asarray(inputs[name], f32).reshape(-1)

    shared = {
        "wkT": np.ascontiguousarray(np.asarray(inputs["Wk"], f32).T.astype(bf)),
        "wvT": np.ascontiguousarray(np.asarray(inputs["Wv"], f32).T.astype(bf)),
        "wrT": np.ascontiguousarray(np.asarray(inputs["Wr"], f32).T.astype(bf)),
        "woT": np.ascontiguousarray(np.asarray(inputs["Wo"], f32).T.astype(bf)),
        "wkfT": np.ascontiguousarray(np.asarray(inputs["Wk_ffn"], f32).T.astype(bf)),
        "wvfT": np.ascontiguousarray(np.asarray(inputs["Wv_ffn"], f32).T.astype(bf)),
        "wrfT": np.ascontiguousarray(np.asarray(inputs["Wr_ffn"], f32).T.astype(bf)),
        "mk": v("att_mix_k"),
        "mv": v("att_mix_v"),
        "mr": v("att_mix_r"),
        "mfk": v("ffn_mix_k"),
        "mfr": v("ffn_mix_r"),
        "lam": np.exp(-np.exp(v("time_decay"))).astype(f32),
        "eu": np.exp(v("time_first")).astype(f32),
        "g1": v("ln1_g"), "b1": v("ln1_b"),
        "g2": v("ln2_g"), "b2": v("ln2_b"),
    }
    return shared


def _ensure_axon_hooks():
    """The agent image's antenv lacks axon_hooks; bass_utils trace=True needs
    it. Install a shim wired to the injected libaxon_pjrt.so NTFF profiler."""
    try:
        import antenv.axon_hooks  # noqa: F401
        return
    except ImportError:
        pass
    import types
    mod = types.ModuleType("antenv.axon_hooks")
    mod._hook = None

    def set_axon_ntff_profile_hook(h):
        mod._hook = h

    def get_axon_ntff_profile_hook():
        return mod._hook

    mod.set_axon_ntff_profile_hook = set_axon_ntff_profile_hook
    mod.get_axon_ntff_profile_hook = get_axon_ntff_profile_hook
    sys.modules["antenv.axon_hooks"] = mod
    import antenv
    antenv.axon_hooks = mod
    try:
        from trn_agent_boot.trn_boot import _ntff_profile_via_ctypes
        so = "/opt/axon/libaxon_pjrt.so"
        if os.path.exists(so):
            mod._hook = _ntff_profile_via_ctypes(so)
    except Exception as e:  # pragma: no cover - degrade to no tracing
        print("ntff hook unavailable:", e)


def kernel(**inputs):
    from concourse import bass_utils

    nc = get_program()
    shared = host_inputs(inputs)
    X = np.asarray(inputs["x"], np.float32)
    in_maps = [dict(shared, x=np.ascontiguousarray(X[b])) for b in range(NCORES)]

    trace = os.environ.get("KERNEL_TRACE", "0") == "1"
    if trace:
        _ensure_axon_hooks()
    res = bass_utils.run_bass_kernel_spmd(
        nc, in_maps, core_ids=list(range(NCORES)), trace=trace)
    kernel._last_exec_ns = res.exec_time_ns
    kernel._last_results = res
    out = np.stack([r["y"] for r in res.results], axis=0)
    return out


kernel._last_exec_ns = None


# revision 22
# speedup vs baseline: 1.0047x; 1.0047x over previous
"""RWKV-4 block (TimeMix + ChannelMix) Trainium2 Bass kernel.

Full inputs in, full outputs out. Sharding: data-parallel over batch B=8
across the 8 NeuronCores (one batch element per core, no collectives).

Per-core layout strategy:
  - LayerNorm + stats in token-major [128 tokens, C] (bn_stats along free dim),
    then PE-transpose to feature-major [128 chan, T] for everything else.
  - token_shift is a free-dim offset view of a front-padded feature-major tile.
  - k/v/r and FFN matmuls in bf16 (fp32 PSUM accumulation).
  - The WKV recurrence A_t = lam*A_{t-1} + e^{k_t} v_t  (and same for B with
    e^{k_t}) runs as a native DVE tensor_tensor_scan per 128-channel block,
    fp32, chained across T-chunks via the `initial` operand.
    y_t = (A_{t-1} + e^u e^{k_t} v_t) / (B_{t-1} + e^u e^{k_t}) -- division
    via ACT Ln+Exp (1/x = exp(-ln x), denominator is strictly positive).
"""

import os
import sys

import numpy as np

if "/opt/trn_rl_repo" not in sys.path:
    sys.path.insert(0, "/opt/trn_rl_repo")

import ml_dtypes

B, T, C, FFN = 8, 2048, 1024, 4096
NCORES = 8
CB = C // 128      # 8 channel blocks
FB = FFN // 128    # 32 ffn blocks
TCA = 512          # TimeMix chunk (tokens)
NCHA = T // TCA    # 4
TCB = 256          # FFN chunk (tokens)
NCHB = T // TCB    # 8
LN_EPS = 1e-5

BF16 = None  # set lazily (mybir.dt.bfloat16)

_CACHE = {}


def _build_program(with_b2r=False):
    import concourse.bass as bass  # noqa: F401
    from concourse import bacc
    import concourse.tile as tile
    import concourse.mybir as mybir
    from concourse.masks import make_identity
    from contextlib import ExitStack

    f32 = mybir.dt.float32
    bf16 = mybir.dt.bfloat16
    AF = mybir.ActivationFunctionType
    OP = mybir.AluOpType

    nc = bacc.Bacc("TRN2", target_bir_lowering=False, debug=False,
                   enable_asserts=False)

    # ---------------- DRAM I/O ----------------
    x_d = nc.dram_tensor("x", [T, C], f32, kind="ExternalInput").ap()
    y_d = nc.dram_tensor("y", [T, C], f32, kind="ExternalOutput").ap()

    wk_d = nc.dram_tensor("wkT", [C, C], bf16, kind="ExternalInput").ap()
    wv_d = nc.dram_tensor("wvT", [C, C], bf16, kind="ExternalInput").ap()
    wr_d = nc.dram_tensor("wrT", [C, C], bf16, kind="ExternalInput").ap()
    wo_d = nc.dram_tensor("woT", [C, C], bf16, kind="ExternalInput").ap()
    wkf_d = nc.dram_tensor("wkfT", [C, FFN], bf16, kind="ExternalInput").ap()
    wvf_d = nc.dram_tensor("wvfT", [FFN, C], bf16, kind="ExternalInput").ap()
    wrf_d = nc.dram_tensor("wrfT", [C, C], bf16, kind="ExternalInput").ap()

    # LN gains are folded into the weights host-side (W <- W diag(g)); LN
    # biases become per-output-channel matmul biases fed through the ACT
    # eviction bias port (bk/bv/brn/bkf) — keeps every ACT op in one
    # function-table set (Copy/Relu/Exp/Ln) so there are no table reloads.
    PARAMS = ["mk", "mv", "mr", "mfk", "mfr", "lam", "eu", "bk", "bv", "brn"]
    par_d = {p: nc.dram_tensor(p, [C], f32, kind="ExternalInput").ap()
             for p in PARAMS}
    bkf_d = nc.dram_tensor("bkf", [FFN], f32, kind="ExternalInput").ap()
    if with_b2r:
        b2r_d = nc.dram_tensor("b2r", [C], f32, kind="ExternalInput").ap()

    with tile.TileContext(nc) as tc, ExitStack() as glob:
        const = glob.enter_context(tc.tile_pool(name="const", bufs=1))
        psT = glob.enter_context(tc.tile_pool(name="psT", bufs=2, space="PSUM"))
        dram = glob.enter_context(tc.tile_pool(name="dram", bufs=1, space="DRAM"))

        # per-channel params as [128, CB] tiles: column cb = channels cb*128..+128
        par = {}
        for p in PARAMS:
            pt = const.tile([128, CB], f32, name=f"p_{p}")
            nc.sync.dma_start(out=pt, in_=par_d[p].rearrange("(a p) -> p a", p=128))
            par[p] = pt
        bkf_t = const.tile([128, FB], f32)
        nc.sync.dma_start(out=bkf_t, in_=bkf_d.rearrange("(a p) -> p a", p=128))
        if with_b2r:
            # r2 bias varies along the free dim in token-major layout; add it
            # via a partition-broadcast [128, C] tile.
            b2r_t = const.tile([128, C], f32)
            import concourse.bass as _bass
            b2r_bcast = _bass.AP(tensor=b2r_d.tensor, offset=b2r_d.offset,
                                 ap=[[0, 128]] + list(b2r_d.ap))
            nc.sync.dma_start(out=b2r_t, in_=b2r_bcast)

        ident = const.tile([128, 128], f32)
        make_identity(nc, ident)

        eps_t = const.tile([128, 1], f32)
        nc.vector.memset(eps_t, LN_EPS)
        one_t = const.tile([128, 1], f32)
        nc.vector.memset(one_t, 1.0)

        Acar = const.tile([128, CB], f32)
        nc.vector.memset(Acar, 0.0)
        Bcar = const.tile([128, CB], f32)
        nc.vector.memset(Bcar, 0.0)
        xlast = const.tile([128, CB], bf16)
        xlast2 = const.tile([128, CB], bf16)

        x2_dram = dram.tile([T, C], f32)

        # ---- helper: LN stats + apply for one [128, C] token tile ----
        def ln_apply(xt, statsp, xnp):
            st6 = statsp.tile([128, 2, 6], f32, name="st6", tag="st6")
            nc.vector.bn_stats(out=st6[:, 0, :], in_=xt[:, 0:512])
            nc.vector.bn_stats(out=st6[:, 1, :], in_=xt[:, 512:1024])
            mv_t = statsp.tile([128, 2], f32, name="mv_t", tag="mv")
            nc.vector.bn_aggr(out=mv_t, in_=st6)
            # rstd = exp(-0.5*ln(var+eps)) — keeps ACT inside the ln/exp
            # table set (Sqrt lives in a different set -> ~2.7us reload).
            ldv = statsp.tile([128, 1], f32, name="ldv", tag="ldv")
            nc.scalar.activation(out=ldv, in_=mv_t[:, 1:2], func=AF.Ln,
                                 bias=eps_t)
            rstd = statsp.tile([128, 1], f32, name="rstd", tag="rstd")
            nc.scalar.activation(out=rstd, in_=ldv, func=AF.Exp, scale=-0.5)
            xnh = xnp.tile([128, C], f32, name="xnh", tag="xnh")
            nc.vector.tensor_scalar(out=xnh, in0=xt, scalar1=mv_t[:, 0:1],
                                    scalar2=rstd, op0=OP.subtract, op1=OP.mult)
            return xnh

        # ================= Phase A: TimeMix =================
        with ExitStack() as ph:
            wp = ph.enter_context(tc.tile_pool(name="wA", bufs=1))
            wk_sb = wp.tile([128, CB, C], bf16)
            wv_sb = wp.tile([128, CB, C], bf16)
            wr_sb = wp.tile([128, CB, C], bf16)
            wo_sb = wp.tile([128, CB, C], bf16)
            for cb in range(CB):
                nc.sync.dma_start(out=wk_sb[:, cb, :], in_=wk_d[cb * 128:(cb + 1) * 128, :])
                nc.sync.dma_start(out=wv_sb[:, cb, :], in_=wv_d[cb * 128:(cb + 1) * 128, :])
                nc.sync.dma_start(out=wr_sb[:, cb, :], in_=wr_d[cb * 128:(cb + 1) * 128, :])
                nc.sync.dma_start(out=wo_sb[:, cb, :], in_=wo_d[cb * 128:(cb + 1) * 128, :])

            xtp = ph.enter_context(tc.tile_pool(name="xtp", bufs=5))
            statsp = ph.enter_context(tc.tile_pool(name="statsA", bufs=4))
            xnp = ph.enter_context(tc.tile_pool(name="xnpA", bufs=2))
            xnTp = ph.enter_context(tc.tile_pool(name="xnTA", bufs=1))
            mixp = ph.enter_context(tc.tile_pool(name="mixA", bufs=1))
            wkvp = ph.enter_context(tc.tile_pool(name="wkv", bufs=2))
            x2p = ph.enter_context(tc.tile_pool(name="x2p", bufs=2))
            psA = ph.enter_context(tc.tile_pool(name="psA", bufs=4, space="PSUM"))
            psO = ph.enter_context(tc.tile_pool(name="psO", bufs=2, space="PSUM"))

            NTT = TCA // 128  # 4 token tiles per chunk

            for ic in range(NCHA):
                t0 = ic * TCA
                xts = []
                for tt in range(NTT):
                    xt = xtp.tile([128, C], f32, name="xt", tag="xt")
                    nc.sync.dma_start(
                        out=xt, in_=x_d[t0 + tt * 128: t0 + (tt + 1) * 128, :])
                    xts.append(xt)

                # feature-major, front-padded by one column for token_shift
                xnT = xnTp.tile([128, CB, TCA + 1], bf16, name="xnT", tag="xnT")
                if ic == 0:
                    nc.gpsimd.memset(xnT[:, :, 0], 0.0)
                else:
                    nc.gpsimd.tensor_copy(out=xnT[:, :, 0], in_=xlast)

                for tt in range(NTT):
                    xnh = ln_apply(xts[tt], statsp, xnp)
                    for cb in range(CB):
                        pt = psT.tile([128, 128], f32, name="pt", tag="pt")
                        nc.tensor.transpose(pt, xnh[:, cb * 128:(cb + 1) * 128], ident)
                        nc.scalar.copy(
                            out=xnT[:, cb, 1 + tt * 128: 1 + (tt + 1) * 128],
                            in_=pt)
                nc.gpsimd.tensor_copy(out=xlast, in_=xnT[:, :, TCA])

                xk_t = mixp.tile([128, CB, TCA], bf16, name="xk_t", tag="xk")
                xv_t = mixp.tile([128, CB, TCA], bf16, name="xv_t", tag="xv")
                xr_t = mixp.tile([128, CB, TCA], bf16, name="xr_t", tag="xr")
                for cb in range(CB):
                    xn_v = xnT[:, cb, 1:TCA + 1]   # xn[t]
                    xx_v = xnT[:, cb, 0:TCA]       # xn[t-1]
                    dd = mixp.tile([128, TCA], bf16, name="dd", tag="dd", bufs=2)
                    nc.vector.tensor_sub(dd, xn_v, xx_v)
                    nc.vector.scalar_tensor_tensor(
                        out=xk_t[:, cb, :], in0=dd, scalar=par["mk"][:, cb:cb + 1],
                        in1=xx_v, op0=OP.mult, op1=OP.add)
                    nc.vector.scalar_tensor_tensor(
                        out=xv_t[:, cb, :], in0=dd, scalar=par["mv"][:, cb:cb + 1],
                        in1=xx_v, op0=OP.mult, op1=OP.add)
                    nc.vector.scalar_tensor_tensor(
                        out=xr_t[:, cb, :], in0=dd, scalar=par["mr"][:, cb:cb + 1],
                        in1=xx_v, op0=OP.mult, op1=OP.add)

                rw_t = wkvp.tile([128, CB, TCA], bf16, name="rw_t", tag="rw")
                for db in range(CB):
                    dsl = slice(db * 128, (db + 1) * 128)
                    psk = psA.tile([128, TCA], f32, name="psk", tag="mmA")
                    for cb in range(CB):
                        nc.tensor.matmul(psk, wk_sb[:, cb, dsl], xk_t[:, cb, :],
                                         start=(cb == 0), stop=(cb == CB - 1))
                    ek = wkvp.tile([128, TCA], f32, name="ek", tag="ek")
                    nc.scalar.activation(out=ek, in_=psk, func=AF.Exp,
                                         bias=par["bk"][:, db:db + 1])

                    psv = psA.tile([128, TCA], f32, name="psv", tag="mmA")
                    for cb in range(CB):
                        nc.tensor.matmul(psv, wv_sb[:, cb, dsl], xv_t[:, cb, :],
                                         start=(cb == 0), stop=(cb == CB - 1))
                    ekv = wkvp.tile([128, TCA], f32, name="ekv", tag="ekv")
                    nc.vector.scalar_tensor_tensor(
                        out=ekv, in0=psv, scalar=par["bv"][:, db:db + 1],
                        in1=ek, op0=OP.add, op1=OP.mult)

                    psr = psA.tile([128, TCA], f32, name="psr", tag="mmA")
                    for cb in range(CB):
                        nc.tensor.matmul(psr, wr_sb[:, cb, dsl], xr_t[:, cb, :],
                                         start=(cb == 0), stop=(cb == CB - 1))
                    # sigmoid(r)*num/den == num / (den*(1+exp(-r))) — avoids
                    # the Sigmoid table set; everything stays in ln/exp.
                    er = wkvp.tile([128, TCA], f32, name="er", tag="sr")
                    nc.scalar.activation(out=er, in_=psr, func=AF.Exp, scale=-1.0,
                                         bias=par["brn"][:, db:db + 1])

                    lam_s = par["lam"][:, db:db + 1]
                    Apad = wkvp.tile([128, TCA + 1], f32, name="Apad", tag="Apad")
                    nc.gpsimd.tensor_copy(out=Apad[:, 0:1], in_=Acar[:, db:db + 1])
                    nc.vector.tensor_tensor_scan(
                        out=Apad[:, 1:TCA + 1], data0=lam_s.broadcast_to([128, TCA]),
                        data1=ekv, initial=Acar[:, db:db + 1],
                        op0=OP.mult, op1=OP.add)
                    nc.gpsimd.tensor_copy(out=Acar[:, db:db + 1],
                                          in_=Apad[:, TCA:TCA + 1])

                    Bpad = wkvp.tile([128, TCA + 1], f32, name="Bpad", tag="Bpad")
                    nc.gpsimd.tensor_copy(out=Bpad[:, 0:1], in_=Bcar[:, db:db + 1])
                    nc.vector.tensor_tensor_scan(
                        out=Bpad[:, 1:TCA + 1], data0=lam_s.broadcast_to([128, TCA]),
                        data1=ek, initial=Bcar[:, db:db + 1],
                        op0=OP.mult, op1=OP.add)
                    nc.gpsimd.tensor_copy(out=Bcar[:, db:db + 1],
                                          in_=Bpad[:, TCA:TCA + 1])

                    eu_s = par["eu"][:, db:db + 1]
                    num = wkvp.tile([128, TCA], f32, name="num", tag="num")
                    nc.vector.scalar_tensor_tensor(
                        out=num, in0=ekv, scalar=eu_s, in1=Apad[:, 0:TCA],
                        op0=OP.mult, op1=OP.add)
                    den = wkvp.tile([128, TCA], f32, name="den", tag="den")
                    nc.vector.scalar_tensor_tensor(
                        out=den, in0=ek, scalar=eu_s, in1=Bpad[:, 0:TCA],
                        op0=OP.mult, op1=OP.add)
                    den2 = wkvp.tile([128, TCA], f32, name="den2", tag="den2")
                    nc.vector.scalar_tensor_tensor(
                        out=den2, in0=er, scalar=1.0, in1=den,
                        op0=OP.add, op1=OP.mult)
                    ldn = wkvp.tile([128, TCA], f32, name="ldn", tag="ldn")
                    nc.scalar.activation(out=ldn, in_=den2, func=AF.Ln)
                    inv = wkvp.tile([128, TCA], f32, name="inv", tag="inv")
                    nc.scalar.activation(out=inv, in_=ldn, func=AF.Exp, scale=-1.0)
                    nc.vector.tensor_mul(rw_t[:, db, :], num, inv)

                # Wo back to token-major + residual
                for tt in range(NTT):
                    for chh in range(2):
                        pso = psO.tile([128, 512], f32, name="pso", tag="mmO")
                        for db in range(CB):
                            nc.tensor.matmul(
                                pso,
                                rw_t[:, db, tt * 128:(tt + 1) * 128],
                                wo_sb[:, db, chh * 512:(chh + 1) * 512],
                                start=(db == 0), stop=(db == CB - 1))
                        x2sb = x2p.tile([128, 512], f32, name="x2sb", tag="x2")
                        nc.vector.tensor_add(
                            x2sb, xts[tt][:, chh * 512:(chh + 1) * 512], pso)
                        nc.sync.dma_start(
                            out=x2_dram[t0 + tt * 128: t0 + (tt + 1) * 128,
                                        chh * 512:(chh + 1) * 512],
                            in_=x2sb)

        # ================= Phase B: ChannelMix =================
        with ExitStack() as ph:
            wp = ph.enter_context(tc.tile_pool(name="wB", bufs=1))
            wkf_sb = wp.tile([128, CB, FFN], bf16)
            for cb in range(CB):
                nc.sync.dma_start(out=wkf_sb[:, cb, :],
                                  in_=wkf_d[cb * 128:(cb + 1) * 128, :])
            wvf_sb = wp.tile([128, FB, C], bf16)
            for fb in range(FB):
                nc.sync.dma_start(out=wvf_sb[:, fb, :],
                                  in_=wvf_d[fb * 128:(fb + 1) * 128, :])
            wrf_sb = wp.tile([128, CB, C], bf16)
            for cb in range(CB):
                nc.sync.dma_start(out=wrf_sb[:, cb, :],
                                  in_=wrf_d[cb * 128:(cb + 1) * 128, :])

            x2tp = ph.enter_context(tc.tile_pool(name="x2tp", bufs=3))
            statsp = ph.enter_context(tc.tile_pool(name="statsB", bufs=4))
            xnp = ph.enter_context(tc.tile_pool(name="xnpB", bufs=2))
            xnTp = ph.enter_context(tc.tile_pool(name="xnTB", bufs=1))
            mixp = ph.enter_context(tc.tile_pool(name="mixB", bufs=1))
            ffp = ph.enter_context(tc.tile_pool(name="ffp", bufs=2))
            psKV = ph.enter_context(tc.tile_pool(name="psKV", bufs=1, space="PSUM"))
            psSm = ph.enter_context(tc.tile_pool(name="psSm", bufs=2, space="PSUM"))

            NTT = TCB // 128  # 2

            for jc in range(NCHB):
                t0 = jc * TCB
                x2ts = []
                for tt in range(NTT):
                    x2t = x2tp.tile([128, C], f32, name="x2t", tag="x2t")
                    nc.sync.dma_start(
                        out=x2t, in_=x2_dram[t0 + tt * 128: t0 + (tt + 1) * 128, :])
                    x2ts.append(x2t)

                xnT = xnTp.tile([128, CB, TCB + 1], bf16, name="xnT2", tag="xnT2")
                if jc == 0:
                    nc.gpsimd.memset(xnT[:, :, 0], 0.0)
                else:
                    nc.gpsimd.tensor_copy(out=xnT[:, :, 0], in_=xlast2)

                for tt in range(NTT):
                    xnh = ln_apply(x2ts[tt], statsp, xnp)
                    for cb in range(CB):
                        pt = psT.tile([128, 128], f32, name="pt2", tag="pt")
                        nc.tensor.transpose(pt, xnh[:, cb * 128:(cb + 1) * 128], ident)
                        nc.scalar.copy(
                            out=xnT[:, cb, 1 + tt * 128: 1 + (tt + 1) * 128],
                            in_=pt)
                nc.gpsimd.tensor_copy(out=xlast2, in_=xnT[:, :, TCB])

                xk_t = mixp.tile([128, CB, TCB], bf16, name="xk2_t", tag="xk2")
                xr_t = mixp.tile([128, CB, TCB], bf16, name="xr2_t", tag="xr2")
                for cb in range(CB):
                    xn_v = xnT[:, cb, 1:TCB + 1]
                    xx_v = xnT[:, cb, 0:TCB]
                    dd = mixp.tile([128, TCB], bf16, name="dd2", tag="dd2", bufs=2)
                    nc.vector.tensor_sub(dd, xn_v, xx_v)
                    nc.vector.scalar_tensor_tensor(
                        out=xk_t[:, cb, :], in0=dd, scalar=par["mfk"][:, cb:cb + 1],
                        in1=xx_v, op0=OP.mult, op1=OP.add)
                    nc.vector.scalar_tensor_tensor(
                        out=xr_t[:, cb, :], in0=dd, scalar=par["mfr"][:, cb:cb + 1],
                        in1=xx_v, op0=OP.mult, op1=OP.add)

                kvps = [[psKV.tile([128, 512], f32, name=f"kv{tt}{chh}",
                                   tag=f"kv{tt}{chh}")
                         for chh in range(2)] for tt in range(NTT)]

                for fb in range(FB):
                    pskf = psSm.tile([128, TCB], f32, name="pskf", tag="sm")
                    for cb in range(CB):
                        nc.tensor.matmul(
                            pskf, wkf_sb[:, cb, fb * 128:(fb + 1) * 128],
                            xk_t[:, cb, :], start=(cb == 0), stop=(cb == CB - 1))
                    kfb = ffp.tile([128, TCB], bf16, name="kfb", tag="kf", bufs=3)
                    nc.scalar.activation(out=kfb, in_=pskf, func=AF.Relu,
                                         bias=bkf_t[:, fb:fb + 1])
                    kfs = ffp.tile([128, TCB], bf16, name="kfs", tag="kfs", bufs=3)
                    nc.vector.tensor_mul(kfs, kfb, kfb)
                    for tt in range(NTT):
                        for chh in range(2):
                            nc.tensor.matmul(
                                kvps[tt][chh],
                                kfs[:, tt * 128:(tt + 1) * 128],
                                wvf_sb[:, fb, chh * 512:(chh + 1) * 512],
                                start=(fb == 0), stop=(fb == FB - 1))

                for tt in range(NTT):
                    for chh in range(2):
                        psr2 = psSm.tile([128, 512], f32, name="psr2", tag="sm")
                        for cb in range(CB):
                            nc.tensor.matmul(
                                psr2,
                                xr_t[:, cb, tt * 128:(tt + 1) * 128],
                                wrf_sb[:, cb, chh * 512:(chh + 1) * 512],
                                start=(cb == 0), stop=(cb == CB - 1))
                        # sigmoid via exp/ln only (same ACT table set):
                        # sig = exp(-ln(1+exp(-r)))
                        if with_b2r:
                            nc.vector.tensor_add(
                                psr2, psr2,
                                b2r_t[:, chh * 512:(chh + 1) * 512])
                        er2 = ffp.tile([128, 512], f32, name="er2", tag="sr2")
                        nc.scalar.activation(out=er2, in_=psr2, func=AF.Exp,
                                             scale=-1.0)
                        ln2t = ffp.tile([128, 512], f32, name="ln2t", tag="ln2t")
                        nc.scalar.activation(out=ln2t, in_=er2, func=AF.Ln,
                                             bias=one_t)
                        sg2 = ffp.tile([128, 512], f32, name="sg2", tag="sg2")
                        nc.scalar.activation(out=sg2, in_=ln2t, func=AF.Exp,
                                             scale=-1.0)
                        tmp = ffp.tile([128, 512], f32, name="tmp", tag="tmp", bufs=1)
                        nc.vector.tensor_mul(tmp, sg2, kvps[tt][chh])
                        x3 = ffp.tile([128, 512], f32, name="x3", tag="x3")
                        nc.vector.tensor_add(
                            x3, tmp, x2ts[tt][:, chh * 512:(chh + 1) * 512])
                        nc.sync.dma_start(
                            out=y_d[t0 + tt * 128: t0 + (tt + 1) * 128,
                                    chh * 512:(chh + 1) * 512],
                            in_=x3)

    nc.compile()
    return nc


def get_program(with_b2r=False):
    key = ("nc", with_b2r)
    if key not in _CACHE:
        _CACHE[key] = _build_program(with_b2r)
    return _CACHE[key]


def host_inputs(inputs):
    """Host-side precompute: transposed bf16 weights + derived per-channel vecs.
    Returns the shared (per-core-identical) part of the in_map."""
    bf = ml_dtypes.bfloat16
    f32 = np.float32

    def v(name):
        return np.asarray(inputs[name], f32).reshape(-1)

    g1, b1 = v("ln1_g"), v("ln1_b")
    g2, b2 = v("ln2_g"), v("ln2_b")
    Wk = np.asarray(inputs["Wk"], f32)
    Wv = np.asarray(inputs["Wv"], f32)
    Wr = np.asarray(inputs["Wr"], f32)
    Wkf = np.asarray(inputs["Wk_ffn"], f32)
    Wrf = np.asarray(inputs["Wr_ffn"], f32)

    def tb(a):
        return np.ascontiguousarray(a.astype(bf))

    shared = {
        # LN gain folded in: row c of W.T scaled by g[c]
        "wkT": tb(Wk.T * g1[:, None]),
        "wvT": tb(Wv.T * g1[:, None]),
        "wrT": tb(Wr.T * g1[:, None]),
        "woT": tb(np.asarray(inputs["Wo"], f32).T),
        "wkfT": tb(Wkf.T * g2[:, None]),
        "wvfT": tb(np.asarray(inputs["Wv_ffn"], f32).T),
        "wrfT": tb(Wrf.T * g2[:, None]),
        "mk": v("att_mix_k"),
        "mv": v("att_mix_v"),
        "mr": v("att_mix_r"),
        "mfk": v("ffn_mix_k"),
        "mfr": v("ffn_mix_r"),
        "lam": np.exp(-np.exp(v("time_decay"))).astype(f32),
        "eu": np.exp(v("time_first")).astype(f32),
        # LN bias folded to per-output-channel matmul bias
        "bk": (Wk @ b1).astype(f32),
        "bv": (Wv @ b1).astype(f32),
        "brn": (-(Wr @ b1)).astype(f32),
        "bkf": (Wkf @ b2).astype(f32),
    }
    b2r = (Wrf @ b2).astype(f32)
    with_b2r = bool(np.any(b2r != 0.0))
    if with_b2r:
        shared["b2r"] = b2r
    return shared, with_b2r


def _ensure_axon_hooks():
    """The agent image's antenv lacks axon_hooks; bass_utils trace=True needs
    it. Install a shim wired to the injected libaxon_pjrt.so NTFF profiler."""
    try:
        import antenv.axon_hooks  # noqa: F401
        return
    except ImportError:
        pass
    import types
    mod = types.ModuleType("antenv.axon_hooks")
    mod._hook = None

    def set_axon_ntff_profile_hook(h):
        mod._hook = h

    def get_axon_ntff_profile_hook():
        return mod._hook

    mod.set_axon_ntff_profile_hook = set_axon_ntff_profile_hook
    mod.get_axon_ntff_profile_hook = get_axon_ntff_profile_hook
    sys.modules["antenv.axon_hooks"] = mod
    import antenv
    antenv.axon_hooks = mod
    try:
        from trn_agent_boot.trn_boot import _ntff_profile_via_ctypes
        so = "/opt/axon/libaxon_pjrt.so"
        if os.path.exists(so):
            mod._hook = _ntff_profile_via_ctypes(so)
    except Exception as e:  # pragma: no cover - degrade to no tracing
        print("ntff hook unavailable:", e)


def kernel(**inputs):
    from concourse import bass_utils

    shared, with_b2r = host_inputs(inputs)
    nc = get_program(with_b2r)
    X = np.asarray(inputs["x"], np.float32)
    in_maps = [dict(shared, x=np.ascontiguousarray(X[b])) for b in range(NCORES)]

    trace = os.environ.get("KERNEL_TRACE", "0") == "1"
    if trace:
        _ensure_axon_hooks()
    res = bass_utils.run_bass_kernel_spmd(
        nc, in_maps, core_ids=list(range(NCORES)), trace=trace)
    kernel._last_exec_ns = res.exec_time_ns
    kernel._last_results = res
    out = np.stack([r["y"] for r in res.results], axis=0)
    return out


kernel._last_exec_ns = None


# revision 23
# speedup vs baseline: 1.1373x; 1.1319x over previous
"""RWKV-4 block (TimeMix + ChannelMix) Trainium2 Bass kernel.

Full inputs in, full outputs out. Sharding: data-parallel over batch B=8
across the 8 NeuronCores (one batch element per core, no collectives).

Per-core layout strategy:
  - LayerNorm + stats in token-major [128 tokens, C] (bn_stats along free dim),
    then PE-transpose to feature-major [128 chan, T] for everything else.
  - token_shift is a free-dim offset view of a front-padded feature-major tile.
  - k/v/r and FFN matmuls in bf16 (fp32 PSUM accumulation).
  - The WKV recurrence A_t = lam*A_{t-1} + e^{k_t} v_t  (and same for B with
    e^{k_t}) runs as a native DVE tensor_tensor_scan per 128-channel block,
    fp32, chained across T-chunks via the `initial` operand.
    y_t = (A_{t-1} + e^u e^{k_t} v_t) / (B_{t-1} + e^u e^{k_t}) -- division
    via ACT Ln+Exp (1/x = exp(-ln x), denominator is strictly positive).
"""

import os
import sys

import numpy as np

if "/opt/trn_rl_repo" not in sys.path:
    sys.path.insert(0, "/opt/trn_rl_repo")

import ml_dtypes

B, T, C, FFN = 8, 2048, 1024, 4096
NCORES = 8
CB = C // 128      # 8 channel blocks
FB = FFN // 128    # 32 ffn blocks
TCA = 512          # TimeMix chunk (tokens)
NCHA = T // TCA    # 4
TCB = 256          # FFN chunk (tokens)
NCHB = T // TCB    # 8
LN_EPS = 1e-5

BF16 = None  # set lazily (mybir.dt.bfloat16)

_CACHE = {}


def _build_program(with_b2r=False):
    import concourse.bass as bass  # noqa: F401
    from concourse import bacc
    import concourse.tile as tile
    import concourse.mybir as mybir
    from concourse.masks import make_identity
    from contextlib import ExitStack

    f32 = mybir.dt.float32
    bf16 = mybir.dt.bfloat16
    AF = mybir.ActivationFunctionType
    OP = mybir.AluOpType

    # Pin ALL activations to the one table set that contains every function
    # we use (ln/exp/relu/copy/identity/square). Without this the table-load
    # pass picks the first set containing each function (exp->exp_and_others,
    # ln->natural_log, ...) and the kernel thrashes ACT_TABLE_LOADs
    # (~2.7us each, measured 188 loads = 241us).
    import concourse.hw_specs as hw_specs
    if not getattr(hw_specs, "_rwkv_patched", False):
        _orig_gat = hw_specs.get_activation_tables

        def _only_lnexp(arch):
            t = _orig_gat(arch)
            keep = "natural_log_exp_and_others"
            return {name: (funcs if name == keep else set())
                    for name, funcs in t.items()}

        hw_specs.get_activation_tables = _only_lnexp
        bacc.get_activation_tables = _only_lnexp
        hw_specs._rwkv_patched = True

    nc = bacc.Bacc("TRN2", target_bir_lowering=False, debug=False,
                   enable_asserts=False)

    # ---------------- DRAM I/O ----------------
    x_d = nc.dram_tensor("x", [T, C], f32, kind="ExternalInput").ap()
    y_d = nc.dram_tensor("y", [T, C], f32, kind="ExternalOutput").ap()

    wk_d = nc.dram_tensor("wkT", [C, C], bf16, kind="ExternalInput").ap()
    wv_d = nc.dram_tensor("wvT", [C, C], bf16, kind="ExternalInput").ap()
    wr_d = nc.dram_tensor("wrT", [C, C], bf16, kind="ExternalInput").ap()
    wo_d = nc.dram_tensor("woT", [C, C], bf16, kind="ExternalInput").ap()
    wkf_d = nc.dram_tensor("wkfT", [C, FFN], bf16, kind="ExternalInput").ap()
    wvf_d = nc.dram_tensor("wvfT", [FFN, C], bf16, kind="ExternalInput").ap()
    wrf_d = nc.dram_tensor("wrfT", [C, C], bf16, kind="ExternalInput").ap()

    # LN gains are folded into the weights host-side (W <- W diag(g)); LN
    # biases become per-output-channel matmul biases fed through the ACT
    # eviction bias port (bk/bv/brn/bkf) — keeps every ACT op in one
    # function-table set (Copy/Relu/Exp/Ln) so there are no table reloads.
    PARAMS = ["mk", "mv", "mr", "mfk", "mfr", "lam", "eu", "bk", "bv", "brn"]
    par_d = {p: nc.dram_tensor(p, [C], f32, kind="ExternalInput").ap()
             for p in PARAMS}
    bkf_d = nc.dram_tensor("bkf", [FFN], f32, kind="ExternalInput").ap()
    if with_b2r:
        b2r_d = nc.dram_tensor("b2r", [C], f32, kind="ExternalInput").ap()

    with tile.TileContext(nc) as tc, ExitStack() as glob:
        const = glob.enter_context(tc.tile_pool(name="const", bufs=1))
        psT = glob.enter_context(tc.tile_pool(name="psT", bufs=2, space="PSUM"))
        dram = glob.enter_context(tc.tile_pool(name="dram", bufs=1, space="DRAM"))

        # per-channel params as [128, CB] tiles: column cb = channels cb*128..+128
        par = {}
        for p in PARAMS:
            pt = const.tile([128, CB], f32, name=f"p_{p}")
            nc.sync.dma_start(out=pt, in_=par_d[p].rearrange("(a p) -> p a", p=128))
            par[p] = pt
        bkf_t = const.tile([128, FB], f32)
        nc.sync.dma_start(out=bkf_t, in_=bkf_d.rearrange("(a p) -> p a", p=128))
        if with_b2r:
            # r2 bias varies along the free dim in token-major layout; add it
            # via a partition-broadcast [128, C] tile.
            b2r_t = const.tile([128, C], f32)
            import concourse.bass as _bass
            b2r_bcast = _bass.AP(tensor=b2r_d.tensor, offset=b2r_d.offset,
                                 ap=[[0, 128]] + list(b2r_d.ap))
            nc.sync.dma_start(out=b2r_t, in_=b2r_bcast)

        ident = const.tile([128, 128], f32)
        make_identity(nc, ident)

        eps_t = const.tile([128, 1], f32)
        nc.vector.memset(eps_t, LN_EPS)
        one_t = const.tile([128, 1], f32)
        nc.vector.memset(one_t, 1.0)

        Acar = const.tile([128, CB], f32)
        nc.vector.memset(Acar, 0.0)
        Bcar = const.tile([128, CB], f32)
        nc.vector.memset(Bcar, 0.0)
        xlast = const.tile([128, CB], bf16)
        xlast2 = const.tile([128, CB], bf16)

        x2_dram = dram.tile([T, C], f32)

        # ---- helper: LN stats + apply for one [128, C] token tile ----
        def ln_apply(xt, statsp, xnp):
            st6 = statsp.tile([128, 2, 6], f32, name="st6", tag="st6")
            nc.vector.bn_stats(out=st6[:, 0, :], in_=xt[:, 0:512])
            nc.vector.bn_stats(out=st6[:, 1, :], in_=xt[:, 512:1024])
            mv_t = statsp.tile([128, 2], f32, name="mv_t", tag="mv")
            nc.vector.bn_aggr(out=mv_t, in_=st6)
            # rstd = exp(-0.5*ln(var+eps)) — keeps ACT inside the ln/exp
            # table set (Sqrt lives in a different set -> ~2.7us reload).
            ldv = statsp.tile([128, 1], f32, name="ldv", tag="ldv")
            nc.scalar.activation(out=ldv, in_=mv_t[:, 1:2], func=AF.Ln,
                                 bias=eps_t)
            rstd = statsp.tile([128, 1], f32, name="rstd", tag="rstd")
            nc.scalar.activation(out=rstd, in_=ldv, func=AF.Exp, scale=-0.5)
            xnh = xnp.tile([128, C], f32, name="xnh", tag="xnh")
            nc.vector.tensor_scalar(out=xnh, in0=xt, scalar1=mv_t[:, 0:1],
                                    scalar2=rstd, op0=OP.subtract, op1=OP.mult)
            return xnh

        # ================= Phase A: TimeMix =================
        with ExitStack() as ph:
            wp = ph.enter_context(tc.tile_pool(name="wA", bufs=1))
            wk_sb = wp.tile([128, CB, C], bf16)
            wv_sb = wp.tile([128, CB, C], bf16)
            wr_sb = wp.tile([128, CB, C], bf16)
            wo_sb = wp.tile([128, CB, C], bf16)
            for cb in range(CB):
                nc.sync.dma_start(out=wk_sb[:, cb, :], in_=wk_d[cb * 128:(cb + 1) * 128, :])
                nc.sync.dma_start(out=wv_sb[:, cb, :], in_=wv_d[cb * 128:(cb + 1) * 128, :])
                nc.sync.dma_start(out=wr_sb[:, cb, :], in_=wr_d[cb * 128:(cb + 1) * 128, :])
                nc.sync.dma_start(out=wo_sb[:, cb, :], in_=wo_d[cb * 128:(cb + 1) * 128, :])

            xtp = ph.enter_context(tc.tile_pool(name="xtp", bufs=5))
            statsp = ph.enter_context(tc.tile_pool(name="statsA", bufs=4))
            xnp = ph.enter_context(tc.tile_pool(name="xnpA", bufs=2))
            xnTp = ph.enter_context(tc.tile_pool(name="xnTA", bufs=1))
            mixp = ph.enter_context(tc.tile_pool(name="mixA", bufs=1))
            wkvp = ph.enter_context(tc.tile_pool(name="wkv", bufs=2))
            x2p = ph.enter_context(tc.tile_pool(name="x2p", bufs=2))
            psA = ph.enter_context(tc.tile_pool(name="psA", bufs=4, space="PSUM"))
            psO = ph.enter_context(tc.tile_pool(name="psO", bufs=2, space="PSUM"))

            NTT = TCA // 128  # 4 token tiles per chunk

            for ic in range(NCHA):
                t0 = ic * TCA
                xts = []
                for tt in range(NTT):
                    xt = xtp.tile([128, C], f32, name="xt", tag="xt")
                    nc.sync.dma_start(
                        out=xt, in_=x_d[t0 + tt * 128: t0 + (tt + 1) * 128, :])
                    xts.append(xt)

                # feature-major, front-padded by one column for token_shift
                xnT = xnTp.tile([128, CB, TCA + 1], bf16, name="xnT", tag="xnT")
                if ic == 0:
                    nc.gpsimd.memset(xnT[:, :, 0], 0.0)
                else:
                    nc.gpsimd.tensor_copy(out=xnT[:, :, 0], in_=xlast)

                for tt in range(NTT):
                    xnh = ln_apply(xts[tt], statsp, xnp)
                    for cb in range(CB):
                        pt = psT.tile([128, 128], f32, name="pt", tag="pt")
                        nc.tensor.transpose(pt, xnh[:, cb * 128:(cb + 1) * 128], ident)
                        nc.scalar.copy(
                            out=xnT[:, cb, 1 + tt * 128: 1 + (tt + 1) * 128],
                            in_=pt)
                nc.gpsimd.tensor_copy(out=xlast, in_=xnT[:, :, TCA])

                xk_t = mixp.tile([128, CB, TCA], bf16, name="xk_t", tag="xk")
                xv_t = mixp.tile([128, CB, TCA], bf16, name="xv_t", tag="xv")
                xr_t = mixp.tile([128, CB, TCA], bf16, name="xr_t", tag="xr")
                for cb in range(CB):
                    xn_v = xnT[:, cb, 1:TCA + 1]   # xn[t]
                    xx_v = xnT[:, cb, 0:TCA]       # xn[t-1]
                    dd = mixp.tile([128, TCA], bf16, name="dd", tag="dd", bufs=2)
                    nc.vector.tensor_sub(dd, xn_v, xx_v)
                    nc.vector.scalar_tensor_tensor(
                        out=xk_t[:, cb, :], in0=dd, scalar=par["mk"][:, cb:cb + 1],
                        in1=xx_v, op0=OP.mult, op1=OP.add)
                    nc.vector.scalar_tensor_tensor(
                        out=xv_t[:, cb, :], in0=dd, scalar=par["mv"][:, cb:cb + 1],
                        in1=xx_v, op0=OP.mult, op1=OP.add)
                    nc.vector.scalar_tensor_tensor(
                        out=xr_t[:, cb, :], in0=dd, scalar=par["mr"][:, cb:cb + 1],
                        in1=xx_v, op0=OP.mult, op1=OP.add)

                rw_t = wkvp.tile([128, CB, TCA], bf16, name="rw_t", tag="rw")
                for db in range(CB):
                    dsl = slice(db * 128, (db + 1) * 128)
                    psk = psA.tile([128, TCA], f32, name="psk", tag="mmA")
                    for cb in range(CB):
                        nc.tensor.matmul(psk, wk_sb[:, cb, dsl], xk_t[:, cb, :],
                                         start=(cb == 0), stop=(cb == CB - 1))
                    ek = wkvp.tile([128, TCA], f32, name="ek", tag="ek")
                    nc.scalar.activation(out=ek, in_=psk, func=AF.Exp,
                                         bias=par["bk"][:, db:db + 1])

                    psv = psA.tile([128, TCA], f32, name="psv", tag="mmA")
                    for cb in range(CB):
                        nc.tensor.matmul(psv, wv_sb[:, cb, dsl], xv_t[:, cb, :],
                                         start=(cb == 0), stop=(cb == CB - 1))
                    ekv = wkvp.tile([128, TCA], f32, name="ekv", tag="ekv")
                    nc.vector.scalar_tensor_tensor(
                        out=ekv, in0=psv, scalar=par["bv"][:, db:db + 1],
                        in1=ek, op0=OP.add, op1=OP.mult)

                    psr = psA.tile([128, TCA], f32, name="psr", tag="mmA")
                    for cb in range(CB):
                        nc.tensor.matmul(psr, wr_sb[:, cb, dsl], xr_t[:, cb, :],
                                         start=(cb == 0), stop=(cb == CB - 1))
                    # sigmoid(r)*num/den == num / (den*(1+exp(-r))) — avoids
                    # the Sigmoid table set; everything stays in ln/exp.
                    er = wkvp.tile([128, TCA], f32, name="er", tag="sr")
                    nc.scalar.activation(out=er, in_=psr, func=AF.Exp, scale=-1.0,
                                         bias=par["brn"][:, db:db + 1])

                    lam_s = par["lam"][:, db:db + 1]
                    Apad = wkvp.tile([128, TCA + 1], f32, name="Apad", tag="Apad")
                    nc.gpsimd.tensor_copy(out=Apad[:, 0:1], in_=Acar[:, db:db + 1])
                    nc.vector.tensor_tensor_scan(
                        out=Apad[:, 1:TCA + 1], data0=lam_s.broadcast_to([128, TCA]),
                        data1=ekv, initial=Acar[:, db:db + 1],
                        op0=OP.mult, op1=OP.add)
                    nc.gpsimd.tensor_copy(out=Acar[:, db:db + 1],
                                          in_=Apad[:, TCA:TCA + 1])

                    Bpad = wkvp.tile([128, TCA + 1], f32, name="Bpad", tag="Bpad")
                    nc.gpsimd.tensor_copy(out=Bpad[:, 0:1], in_=Bcar[:, db:db + 1])
                    nc.vector.tensor_tensor_scan(
                        out=Bpad[:, 1:TCA + 1], data0=lam_s.broadcast_to([128, TCA]),
                        data1=ek, initial=Bcar[:, db:db + 1],
                        op0=OP.mult, op1=OP.add)
                    nc.gpsimd.tensor_copy(out=Bcar[:, db:db + 1],
                                          in_=Bpad[:, TCA:TCA + 1])

                    eu_s = par["eu"][:, db:db + 1]
                    num = wkvp.tile([128, TCA], f32, name="num", tag="num")
                    nc.vector.scalar_tensor_tensor(
                        out=num, in0=ekv, scalar=eu_s, in1=Apad[:, 0:TCA],
                        op0=OP.mult, op1=OP.add)
                    den = wkvp.tile([128, TCA], f32, name="den", tag="den")
                    nc.vector.scalar_tensor_tensor(
                        out=den, in0=ek, scalar=eu_s, in1=Bpad[:, 0:TCA],
                        op0=OP.mult, op1=OP.add)
                    den2 = wkvp.tile([128, TCA], f32, name="den2", tag="den2")
                    nc.vector.scalar_tensor_tensor(
                        out=den2, in0=er, scalar=1.0, in1=den,
                        op0=OP.add, op1=OP.mult)
                    ldn = wkvp.tile([128, TCA], f32, name="ldn", tag="ldn")
                    nc.scalar.activation(out=ldn, in_=den2, func=AF.Ln)
                    inv = wkvp.tile([128, TCA], f32, name="inv", tag="inv")
                    nc.scalar.activation(out=inv, in_=ldn, func=AF.Exp, scale=-1.0)
                    nc.vector.tensor_mul(rw_t[:, db, :], num, inv)

                # Wo back to token-major + residual
                for tt in range(NTT):
                    for chh in range(2):
                        pso = psO.tile([128, 512], f32, name="pso", tag="mmO")
                        for db in range(CB):
                            nc.tensor.matmul(
                                pso,
                                rw_t[:, db, tt * 128:(tt + 1) * 128],
                                wo_sb[:, db, chh * 512:(chh + 1) * 512],
                                start=(db == 0), stop=(db == CB - 1))
                        x2sb = x2p.tile([128, 512], f32, name="x2sb", tag="x2")
                        nc.vector.tensor_add(
                            x2sb, xts[tt][:, chh * 512:(chh + 1) * 512], pso)
                        nc.sync.dma_start(
                            out=x2_dram[t0 + tt * 128: t0 + (tt + 1) * 128,
                                        chh * 512:(chh + 1) * 512],
                            in_=x2sb)

        # ================= Phase B: ChannelMix =================
        with ExitStack() as ph:
            wp = ph.enter_context(tc.tile_pool(name="wB", bufs=1))
            wkf_sb = wp.tile([128, CB, FFN], bf16)
            for cb in range(CB):
                nc.sync.dma_start(out=wkf_sb[:, cb, :],
                                  in_=wkf_d[cb * 128:(cb + 1) * 128, :])
            wvf_sb = wp.tile([128, FB, C], bf16)
            for fb in range(FB):
                nc.sync.dma_start(out=wvf_sb[:, fb, :],
                                  in_=wvf_d[fb * 128:(fb + 1) * 128, :])
            wrf_sb = wp.tile([128, CB, C], bf16)
            for cb in range(CB):
                nc.sync.dma_start(out=wrf_sb[:, cb, :],
                                  in_=wrf_d[cb * 128:(cb + 1) * 128, :])

            x2tp = ph.enter_context(tc.tile_pool(name="x2tp", bufs=3))
            statsp = ph.enter_context(tc.tile_pool(name="statsB", bufs=4))
            xnp = ph.enter_context(tc.tile_pool(name="xnpB", bufs=2))
            xnTp = ph.enter_context(tc.tile_pool(name="xnTB", bufs=1))
            mixp = ph.enter_context(tc.tile_pool(name="mixB", bufs=1))
            ffp = ph.enter_context(tc.tile_pool(name="ffp", bufs=2))
            psKV = ph.enter_context(tc.tile_pool(name="psKV", bufs=1, space="PSUM"))
            psSm = ph.enter_context(tc.tile_pool(name="psSm", bufs=2, space="PSUM"))

            NTT = TCB // 128  # 2

            for jc in range(NCHB):
                t0 = jc * TCB
                x2ts = []
                for tt in range(NTT):
                    x2t = x2tp.tile([128, C], f32, name="x2t", tag="x2t")
                    nc.sync.dma_start(
                        out=x2t, in_=x2_dram[t0 + tt * 128: t0 + (tt + 1) * 128, :])
                    x2ts.append(x2t)

                xnT = xnTp.tile([128, CB, TCB + 1], bf16, name="xnT2", tag="xnT2")
                if jc == 0:
                    nc.gpsimd.memset(xnT[:, :, 0], 0.0)
                else:
                    nc.gpsimd.tensor_copy(out=xnT[:, :, 0], in_=xlast2)

                for tt in range(NTT):
                    xnh = ln_apply(x2ts[tt], statsp, xnp)
                    for cb in range(CB):
                        pt = psT.tile([128, 128], f32, name="pt2", tag="pt")
                        nc.tensor.transpose(pt, xnh[:, cb * 128:(cb + 1) * 128], ident)
                        nc.scalar.copy(
                            out=xnT[:, cb, 1 + tt * 128: 1 + (tt + 1) * 128],
                            in_=pt)
                nc.gpsimd.tensor_copy(out=xlast2, in_=xnT[:, :, TCB])

                xk_t = mixp.tile([128, CB, TCB], bf16, name="xk2_t", tag="xk2")
                xr_t = mixp.tile([128, CB, TCB], bf16, name="xr2_t", tag="xr2")
                for cb in range(CB):
                    xn_v = xnT[:, cb, 1:TCB + 1]
                    xx_v = xnT[:, cb, 0:TCB]
                    dd = mixp.tile([128, TCB], bf16, name="dd2", tag="dd2", bufs=2)
                    nc.vector.tensor_sub(dd, xn_v, xx_v)
                    nc.vector.scalar_tensor_tensor(
                        out=xk_t[:, cb, :], in0=dd, scalar=par["mfk"][:, cb:cb + 1],
                        in1=xx_v, op0=OP.mult, op1=OP.add)
                    nc.vector.scalar_tensor_tensor(
                        out=xr_t[:, cb, :], in0=dd, scalar=par["mfr"][:, cb:cb + 1],
                        in1=xx_v, op0=OP.mult, op1=OP.add)

                kvps = [[psKV.tile([128, 512], f32, name=f"kv{tt}{chh}",
                                   tag=f"kv{tt}{chh}")
                         for chh in range(2)] for tt in range(NTT)]

                for fb in range(FB):
                    pskf = psSm.tile([128, TCB], f32, name="pskf", tag="sm")
                    for cb in range(CB):
                        nc.tensor.matmul(
                            pskf, wkf_sb[:, cb, fb * 128:(fb + 1) * 128],
                            xk_t[:, cb, :], start=(cb == 0), stop=(cb == CB - 1))
                    kfb = ffp.tile([128, TCB], bf16, name="kfb", tag="kf", bufs=3)
                    nc.scalar.activation(out=kfb, in_=pskf, func=AF.Relu,
                                         bias=bkf_t[:, fb:fb + 1])
                    kfs = ffp.tile([128, TCB], bf16, name="kfs", tag="kfs", bufs=3)
                    nc.vector.tensor_mul(kfs, kfb, kfb)
                    for tt in range(NTT):
                        for chh in range(2):
                            nc.tensor.matmul(
                                kvps[tt][chh],
                                kfs[:, tt * 128:(tt + 1) * 128],
                                wvf_sb[:, fb, chh * 512:(chh + 1) * 512],
                                start=(fb == 0), stop=(fb == FB - 1))

                for tt in range(NTT):
                    for chh in range(2):
                        psr2 = psSm.tile([128, 512], f32, name="psr2", tag="sm")
                        for cb in range(CB):
                            nc.tensor.matmul(
                                psr2,
                                xr_t[:, cb, tt * 128:(tt + 1) * 128],
                                wrf_sb[:, cb, chh * 512:(chh + 1) * 512],
                                start=(cb == 0), stop=(cb == CB - 1))
                        # sigmoid via exp/ln only (same ACT table set):
                        # sig = exp(-ln(1+exp(-r)))
                        if with_b2r:
                            nc.vector.tensor_add(
                                psr2, psr2,
                                b2r_t[:, chh * 512:(chh + 1) * 512])
                        er2 = ffp.tile([128, 512], f32, name="er2", tag="sr2")
                        nc.scalar.activation(out=er2, in_=psr2, func=AF.Exp,
                                             scale=-1.0)
                        ln2t = ffp.tile([128, 512], f32, name="ln2t", tag="ln2t")
                        nc.scalar.activation(out=ln2t, in_=er2, func=AF.Ln,
                                             bias=one_t)
                        sg2 = ffp.tile([128, 512], f32, name="sg2", tag="sg2")
                        nc.scalar.activation(out=sg2, in_=ln2t, func=AF.Exp,
                                             scale=-1.0)
                        tmp = ffp.tile([128, 512], f32, name="tmp", tag="tmp", bufs=1)
                        nc.vector.tensor_mul(tmp, sg2, kvps[tt][chh])
                        x3 = ffp.tile([128, 512], f32, name="x3", tag="x3")
                        nc.vector.tensor_add(
                            x3, tmp, x2ts[tt][:, chh * 512:(chh + 1) * 512])
                        nc.sync.dma_start(
                            out=y_d[t0 + tt * 128: t0 + (tt + 1) * 128,
                                    chh * 512:(chh + 1) * 512],
                            in_=x3)

    nc.compile()
    return nc


def get_program(with_b2r=False):
    key = ("nc", with_b2r)
    if key not in _CACHE:
        _CACHE[key] = _build_program(with_b2r)
    return _CACHE[key]


def host_inputs(inputs):
    """Host-side precompute: transposed bf16 weights + derived per-channel vecs.
    Returns the shared (per-core-identical) part of the in_map."""
    bf = ml_dtypes.bfloat16
    f32 = np.float32

    def v(name):
        return np.asarray(inputs[name], f32).reshape(-1)

    g1, b1 = v("ln1_g"), v("ln1_b")
    g2, b2 = v("ln2_g"), v("ln2_b")
    Wk = np.asarray(inputs["Wk"], f32)
    Wv = np.asarray(inputs["Wv"], f32)
    Wr = np.asarray(inputs["Wr"], f32)
    Wkf = np.asarray(inputs["Wk_ffn"], f32)
    Wrf = np.asarray(inputs["Wr_ffn"], f32)

    def tb(a):
        return np.ascontiguousarray(a.astype(bf))

    shared = {
        # LN gain folded in: row c of W.T scaled by g[c]
        "wkT": tb(Wk.T * g1[:, None]),
        "wvT": tb(Wv.T * g1[:, None]),
        "wrT": tb(Wr.T * g1[:, None]),
        "woT": tb(np.asarray(inputs["Wo"], f32).T),
        "wkfT": tb(Wkf.T * g2[:, None]),
        "wvfT": tb(np.asarray(inputs["Wv_ffn"], f32).T),
        "wrfT": tb(Wrf.T * g2[:, None]),
        "mk": v("att_mix_k"),
        "mv": v("att_mix_v"),
        "mr": v("att_mix_r"),
        "mfk": v("ffn_mix_k"),
        "mfr": v("ffn_mix_r"),
        "lam": np.exp(-np.exp(v("time_decay"))).astype(f32),
        "eu": np.exp(v("time_first")).astype(f32),
        # LN bias folded to per-output-channel matmul bias
        "bk": (Wk @ b1).astype(f32),
        "bv": (Wv @ b1).astype(f32),
        "brn": (-(Wr @ b1)).astype(f32),
        "bkf": (Wkf @ b2).astype(f32),
    }
    b2r = (Wrf @ b2).astype(f32)
    with_b2r = bool(np.any(b2r != 0.0))
    if with_b2r:
        shared["b2r"] = b2r
    return shared, with_b2r


def _ensure_axon_hooks():
    """The agent image's antenv lacks axon_hooks; bass_utils trace=True needs
    it. Install a shim wired to the injected libaxon_pjrt.so NTFF profiler."""
    try:
        import antenv.axon_hooks  # noqa: F401
        return
    except ImportError:
        pass
    import types
    mod = types.ModuleType("antenv.axon_hooks")
    mod._hook = None

    def set_axon_ntff_profile_hook(h):
        mod._hook = h

    def get_axon_ntff_profile_hook():
        return mod._hook

    mod.set_axon_ntff_profile_hook = set_axon_ntff_profile_hook
    mod.get_axon_ntff_profile_hook = get_axon_ntff_profile_hook
    sys.modules["antenv.axon_hooks"] = mod
    import antenv
    antenv.axon_hooks = mod
    try:
        from trn_agent_boot.trn_boot import _ntff_profile_via_ctypes
        so = "/opt/axon/libaxon_pjrt.so"
        if os.path.exists(so):
            mod._hook = _ntff_profile_via_ctypes(so)
    except Exception as e:  # pragma: no cover - degrade to no tracing
        print("ntff hook unavailable:", e)


def kernel(**inputs):
    from concourse import bass_utils

    shared, with_b2r = host_inputs(inputs)
    nc = get_program(with_b2r)
    X = np.asarray(inputs["x"], np.float32)
    in_maps = [dict(shared, x=np.ascontiguousarray(X[b])) for b in range(NCORES)]

    trace = os.environ.get("KERNEL_TRACE", "0") == "1"
    if trace:
        _ensure_axon_hooks()
    res = bass_utils.run_bass_kernel_spmd(
        nc, in_maps, core_ids=list(range(NCORES)), trace=trace)
    kernel._last_exec_ns = res.exec_time_ns
    kernel._last_results = res
    out = np.stack([r["y"] for r in res.results], axis=0)
    return out


kernel._last_exec_ns = None


# revision 34
# speedup vs baseline: 1.1747x; 1.0329x over previous
"""RWKV-4 block (TimeMix + ChannelMix) Trainium2 Bass kernel.

Full inputs in, full outputs out. Sharding: data-parallel over batch B=8
across the 8 NeuronCores (one batch element per core, no collectives).

Per-core layout strategy:
  - LayerNorm + stats in token-major [128 tokens, C] (bn_stats along free dim),
    then PE-transpose to feature-major [128 chan, T] for everything else.
  - token_shift is a free-dim offset view of a front-padded feature-major tile.
  - k/v/r and FFN matmuls in bf16 (fp32 PSUM accumulation).
  - The WKV recurrence A_t = lam*A_{t-1} + e^{k_t} v_t  (and same for B with
    e^{k_t}) runs as a native DVE tensor_tensor_scan per 128-channel block,
    fp32, chained across T-chunks via the `initial` operand.
    y_t = (A_{t-1} + e^u e^{k_t} v_t) / (B_{t-1} + e^u e^{k_t}) -- division
    via ACT Ln+Exp (1/x = exp(-ln x), denominator is strictly positive).
"""

import os
import sys

import numpy as np

if "/opt/trn_rl_repo" not in sys.path:
    sys.path.insert(0, "/opt/trn_rl_repo")

import ml_dtypes

B, T, C, FFN = 8, 2048, 1024, 4096
NCORES = 8
CB = C // 128      # 8 channel blocks
FB = FFN // 128    # 32 ffn blocks
TCA = 512          # TimeMix chunk (tokens)
NCHA = T // TCA    # 4
TCB = 256          # FFN chunk (tokens)
NCHB = T // TCB    # 8
LN_EPS = 1e-5

BF16 = None  # set lazily (mybir.dt.bfloat16)

_CACHE = {}


def _build_program(with_b2r=False):
    import concourse.bass as bass  # noqa: F401
    from concourse import bacc
    import concourse.tile as tile
    import concourse.mybir as mybir
    from concourse.masks import make_identity
    from contextlib import ExitStack

    f32 = mybir.dt.float32
    bf16 = mybir.dt.bfloat16
    AF = mybir.ActivationFunctionType
    OP = mybir.AluOpType

    # Pin ALL activations to the one table set that contains every function
    # we use (ln/exp/relu/copy/identity/square). Without this the table-load
    # pass picks the first set containing each function (exp->exp_and_others,
    # ln->natural_log, ...) and the kernel thrashes ACT_TABLE_LOADs
    # (~2.7us each, measured 188 loads = 241us).
    import concourse.hw_specs as hw_specs
    if not getattr(hw_specs, "_rwkv_patched", False):
        _orig_gat = hw_specs.get_activation_tables

        def _only_lnexp(arch):
            t = _orig_gat(arch)
            keep = "natural_log_exp_and_others"
            return {name: (funcs if name == keep else set())
                    for name, funcs in t.items()}

        hw_specs.get_activation_tables = _only_lnexp
        bacc.get_activation_tables = _only_lnexp
        hw_specs._rwkv_patched = True

    nc = bacc.Bacc("TRN2", target_bir_lowering=False, debug=False,
                   enable_asserts=False)

    # ---------------- DRAM I/O ----------------
    x_d = nc.dram_tensor("x", [T, C], f32, kind="ExternalInput").ap()
    y_d = nc.dram_tensor("y", [T, C], f32, kind="ExternalOutput").ap()

    wk_d = nc.dram_tensor("wkT", [C, C], bf16, kind="ExternalInput").ap()
    wv_d = nc.dram_tensor("wvT", [C, C], bf16, kind="ExternalInput").ap()
    wr_d = nc.dram_tensor("wrT", [C, C], bf16, kind="ExternalInput").ap()
    wo_d = nc.dram_tensor("woT", [C, C], bf16, kind="ExternalInput").ap()
    wkf_d = nc.dram_tensor("wkfT", [C, FFN], bf16, kind="ExternalInput").ap()
    wvf_d = nc.dram_tensor("wvfT", [FFN, C], bf16, kind="ExternalInput").ap()
    wrf_d = nc.dram_tensor("wrfT", [C, C], bf16, kind="ExternalInput").ap()

    # LN gains are folded into the weights host-side (W <- W diag(g)); LN
    # biases become per-output-channel matmul biases fed through the ACT
    # eviction bias port (bk/bv/brn/bkf) — keeps every ACT op in one
    # function-table set (Copy/Relu/Exp/Ln) so there are no table reloads.
    PARAMS = ["mk", "mv", "mr", "mfk", "mfr", "lam", "eu", "bk", "bv", "brn"]
    par_d = {p: nc.dram_tensor(p, [C], f32, kind="ExternalInput").ap()
             for p in PARAMS}
    bkf_d = nc.dram_tensor("bkf", [FFN], f32, kind="ExternalInput").ap()
    if with_b2r:
        b2r_d = nc.dram_tensor("b2r", [C], f32, kind="ExternalInput").ap()

    with tile.TileContext(nc) as tc, ExitStack() as glob:
        const = glob.enter_context(tc.tile_pool(name="const", bufs=1))
        psT = glob.enter_context(tc.tile_pool(name="psT", bufs=2, space="PSUM"))
        dram = glob.enter_context(tc.tile_pool(name="dram", bufs=1, space="DRAM"))

        # per-channel params as [128, CB] tiles: column cb = channels cb*128..+128
        par = {}
        for p in PARAMS:
            pt = const.tile([128, CB], f32, name=f"p_{p}")
            nc.sync.dma_start(out=pt, in_=par_d[p].rearrange("(a p) -> p a", p=128))
            par[p] = pt
        bkf_t = const.tile([128, FB], f32)
        nc.sync.dma_start(out=bkf_t, in_=bkf_d.rearrange("(a p) -> p a", p=128))
        if with_b2r:
            # r2 bias varies along the free dim in token-major layout; add it
            # via a partition-broadcast [128, C] tile.
            b2r_t = const.tile([128, C], f32)
            import concourse.bass as _bass
            b2r_bcast = _bass.AP(tensor=b2r_d.tensor, offset=b2r_d.offset,
                                 ap=[[0, 128]] + list(b2r_d.ap))
            nc.sync.dma_start(out=b2r_t, in_=b2r_bcast)

        ident = const.tile([128, 128], f32)
        make_identity(nc, ident)

        eps_t = const.tile([128, 1], f32)
        nc.vector.memset(eps_t, LN_EPS)
        one_t = const.tile([128, 1], f32)
        nc.vector.memset(one_t, 1.0)

        Acar = const.tile([128, CB], f32)
        nc.vector.memset(Acar, 0.0)
        Bcar = const.tile([128, CB], f32)
        nc.vector.memset(Bcar, 0.0)
        xlast = const.tile([128, CB], bf16)
        xlast2 = const.tile([128, CB], bf16)

        x2_dram = dram.tile([T, C], f32)

        # ---- helper: LN stats + apply for one [128, C] token tile ----
        def ln_apply(xt, statsp, xnp):
            st6 = statsp.tile([128, 2, 6], f32, name="st6", tag="st6")
            nc.vector.bn_stats(out=st6[:, 0, :], in_=xt[:, 0:512])
            nc.vector.bn_stats(out=st6[:, 1, :], in_=xt[:, 512:1024])
            mv_t = statsp.tile([128, 2], f32, name="mv_t", tag="mv")
            nc.vector.bn_aggr(out=mv_t, in_=st6)
            # rstd = exp(-0.5*ln(var+eps)) — keeps ACT inside the ln/exp
            # table set (Sqrt lives in a different set -> ~2.7us reload).
            ldv = statsp.tile([128, 1], f32, name="ldv", tag="ldv")
            nc.scalar.activation(out=ldv, in_=mv_t[:, 1:2], func=AF.Ln,
                                 bias=eps_t)
            rstd = statsp.tile([128, 1], f32, name="rstd", tag="rstd")
            nc.scalar.activation(out=rstd, in_=ldv, func=AF.Exp, scale=-0.5)
            xnh = xnp.tile([128, C], f32, name="xnh", tag="xnh")
            nc.vector.tensor_scalar(out=xnh, in0=xt, scalar1=mv_t[:, 0:1],
                                    scalar2=rstd, op0=OP.subtract, op1=OP.mult)
            return xnh

        # ================= Phase A: TimeMix =================
        with ExitStack() as ph:
            wp = ph.enter_context(tc.tile_pool(name="wA", bufs=1))
            wk_sb = wp.tile([128, CB, C], bf16)
            wv_sb = wp.tile([128, CB, C], bf16)
            wr_sb = wp.tile([128, CB, C], bf16)
            wo_sb = wp.tile([128, CB, C], bf16)
            for cb in range(CB):
                nc.sync.dma_start(out=wk_sb[:, cb, :], in_=wk_d[cb * 128:(cb + 1) * 128, :])
                nc.sync.dma_start(out=wv_sb[:, cb, :], in_=wv_d[cb * 128:(cb + 1) * 128, :])
                nc.sync.dma_start(out=wr_sb[:, cb, :], in_=wr_d[cb * 128:(cb + 1) * 128, :])
                nc.sync.dma_start(out=wo_sb[:, cb, :], in_=wo_d[cb * 128:(cb + 1) * 128, :])

            xtp = ph.enter_context(tc.tile_pool(name="xtp", bufs=5))
            statsp = ph.enter_context(tc.tile_pool(name="statsA", bufs=4))
            xnp = ph.enter_context(tc.tile_pool(name="xnpA", bufs=2))
            xnTp = ph.enter_context(tc.tile_pool(name="xnTA", bufs=2))
            mixp = ph.enter_context(tc.tile_pool(name="mixA", bufs=2))
            wkvp = ph.enter_context(tc.tile_pool(name="wkv", bufs=2))
            x2p = ph.enter_context(tc.tile_pool(name="x2p", bufs=2))
            psA = ph.enter_context(tc.tile_pool(name="psA", bufs=4, space="PSUM"))
            psO = ph.enter_context(tc.tile_pool(name="psO", bufs=2, space="PSUM"))

            NTT = TCA // 128  # 4 token tiles per chunk

            for ic in range(NCHA):
                _sid, _ = nc.enter_named_scope(f"A{ic}", False)
                t0 = ic * TCA
                xts = []
                for tt in range(NTT):
                    xt = xtp.tile([128, C], f32, name="xt", tag="xt")
                    nc.sync.dma_start(
                        out=xt, in_=x_d[t0 + tt * 128: t0 + (tt + 1) * 128, :])
                    xts.append(xt)

                # feature-major, front-padded by one column for token_shift
                xnT = xnTp.tile([128, CB, TCA + 1], bf16, name="xnT", tag="xnT")
                if ic == 0:
                    nc.gpsimd.memset(xnT[:, :, 0], 0.0)
                else:
                    nc.gpsimd.tensor_copy(out=xnT[:, :, 0], in_=xlast)

                for tt in range(NTT):
                    xnh = ln_apply(xts[tt], statsp, xnp)
                    for cb in range(CB):
                        pt = psT.tile([128, 128], f32, name="pt", tag="pt")
                        nc.tensor.transpose(pt, xnh[:, cb * 128:(cb + 1) * 128], ident)
                        nc.scalar.copy(
                            out=xnT[:, cb, 1 + tt * 128: 1 + (tt + 1) * 128],
                            in_=pt)
                nc.gpsimd.tensor_copy(out=xlast, in_=xnT[:, :, TCA])

                xk_t = mixp.tile([128, CB, TCA], bf16, name="xk_t", tag="xk")
                xv_t = mixp.tile([128, CB, TCA], bf16, name="xv_t", tag="xv")
                xr_t = mixp.tile([128, CB, TCA], bf16, name="xr_t", tag="xr")
                for cb in range(CB):
                    xn_v = xnT[:, cb, 1:TCA + 1]   # xn[t]
                    xx_v = xnT[:, cb, 0:TCA]       # xn[t-1]
                    dd = mixp.tile([128, TCA], bf16, name="dd", tag="dd", bufs=2)
                    nc.vector.tensor_sub(dd, xn_v, xx_v)
                    nc.vector.scalar_tensor_tensor(
                        out=xk_t[:, cb, :], in0=dd, scalar=par["mk"][:, cb:cb + 1],
                        in1=xx_v, op0=OP.mult, op1=OP.add)
                    nc.vector.scalar_tensor_tensor(
                        out=xv_t[:, cb, :], in0=dd, scalar=par["mv"][:, cb:cb + 1],
                        in1=xx_v, op0=OP.mult, op1=OP.add)
                    nc.vector.scalar_tensor_tensor(
                        out=xr_t[:, cb, :], in0=dd, scalar=par["mr"][:, cb:cb + 1],
                        in1=xx_v, op0=OP.mult, op1=OP.add)

                rw_t = wkvp.tile([128, CB, TCA], bf16, name="rw_t", tag="rw")
                for db in range(CB):
                    dsl = slice(db * 128, (db + 1) * 128)
                    psk = psA.tile([128, TCA], f32, name="psk", tag="mmA")
                    for cb in range(CB):
                        nc.tensor.matmul(psk, wk_sb[:, cb, dsl], xk_t[:, cb, :],
                                         start=(cb == 0), stop=(cb == CB - 1))
                    ek = wkvp.tile([128, TCA], f32, name="ek", tag="ek")
                    nc.scalar.activation(out=ek, in_=psk, func=AF.Exp,
                                         bias=par["bk"][:, db:db + 1])

                    psv = psA.tile([128, TCA], f32, name="psv", tag="mmA")
                    for cb in range(CB):
                        nc.tensor.matmul(psv, wv_sb[:, cb, dsl], xv_t[:, cb, :],
                                         start=(cb == 0), stop=(cb == CB - 1))
                    ekv = wkvp.tile([128, TCA], f32, name="ekv", tag="ekv")
                    nc.vector.scalar_tensor_tensor(
                        out=ekv, in0=psv, scalar=par["bv"][:, db:db + 1],
                        in1=ek, op0=OP.add, op1=OP.mult)

                    psr = psA.tile([128, TCA], f32, name="psr", tag="mmA")
                    for cb in range(CB):
                        nc.tensor.matmul(psr, wr_sb[:, cb, dsl], xr_t[:, cb, :],
                                         start=(cb == 0), stop=(cb == CB - 1))
                    # sigmoid(r)*num/den == num / (den*(1+exp(-r))) — avoids
                    # the Sigmoid table set; everything stays in ln/exp.
                    er = wkvp.tile([128, TCA], f32, name="er", tag="sr")
                    nc.scalar.activation(out=er, in_=psr, func=AF.Exp, scale=-1.0,
                                         bias=par["brn"][:, db:db + 1])

                    lam_s = par["lam"][:, db:db + 1]
                    Apad = wkvp.tile([128, TCA + 1], f32, name="Apad", tag="Apad")
                    nc.gpsimd.tensor_copy(out=Apad[:, 0:1], in_=Acar[:, db:db + 1])
                    nc.vector.tensor_tensor_scan(
                        out=Apad[:, 1:TCA + 1], data0=lam_s.broadcast_to([128, TCA]),
                        data1=ekv, initial=Acar[:, db:db + 1],
                        op0=OP.mult, op1=OP.add)
                    nc.gpsimd.tensor_copy(out=Acar[:, db:db + 1],
                                          in_=Apad[:, TCA:TCA + 1])

                    Bpad = wkvp.tile([128, TCA + 1], f32, name="Bpad", tag="Bpad")
                    nc.gpsimd.tensor_copy(out=Bpad[:, 0:1], in_=Bcar[:, db:db + 1])
                    nc.vector.tensor_tensor_scan(
                        out=Bpad[:, 1:TCA + 1], data0=lam_s.broadcast_to([128, TCA]),
                        data1=ek, initial=Bcar[:, db:db + 1],
                        op0=OP.mult, op1=OP.add)
                    nc.gpsimd.tensor_copy(out=Bcar[:, db:db + 1],
                                          in_=Bpad[:, TCA:TCA + 1])

                    # In-place tail to keep the wkv pool small:
                    #   num -> ekv, den -> ek, (1+er)*den -> er, ln/exp -> er
                    eu_s = par["eu"][:, db:db + 1]
                    nc.vector.scalar_tensor_tensor(
                        out=ekv, in0=ekv, scalar=eu_s, in1=Apad[:, 0:TCA],
                        op0=OP.mult, op1=OP.add)
                    nc.vector.scalar_tensor_tensor(
                        out=ek, in0=ek, scalar=eu_s, in1=Bpad[:, 0:TCA],
                        op0=OP.mult, op1=OP.add)
                    nc.vector.scalar_tensor_tensor(
                        out=er, in0=er, scalar=1.0, in1=ek,
                        op0=OP.add, op1=OP.mult)
                    nc.scalar.activation(out=er, in_=er, func=AF.Ln)
                    nc.scalar.activation(out=er, in_=er, func=AF.Exp, scale=-1.0)
                    nc.vector.tensor_mul(rw_t[:, db, :], ekv, er)

                # Wo back to token-major + residual
                for tt in range(NTT):
                    for chh in range(2):
                        pso = psO.tile([128, 512], f32, name="pso", tag="mmO")
                        for db in range(CB):
                            nc.tensor.matmul(
                                pso,
                                rw_t[:, db, tt * 128:(tt + 1) * 128],
                                wo_sb[:, db, chh * 512:(chh + 1) * 512],
                                start=(db == 0), stop=(db == CB - 1))
                        x2sb = x2p.tile([128, 512], f32, name="x2sb", tag="x2")
                        nc.vector.tensor_add(
                            x2sb, xts[tt][:, chh * 512:(chh + 1) * 512], pso)
                        nc.sync.dma_start(
                            out=x2_dram[t0 + tt * 128: t0 + (tt + 1) * 128,
                                        chh * 512:(chh + 1) * 512],
                            in_=x2sb)
                nc.leave_named_scope(f"A{ic}", _sid, False)

        # ================= Phase B: ChannelMix =================
        with ExitStack() as ph:
            wp = ph.enter_context(tc.tile_pool(name="wB", bufs=1))
            wkf_sb = wp.tile([128, CB, FFN], bf16)
            for cb in range(CB):
                nc.sync.dma_start(out=wkf_sb[:, cb, :],
                                  in_=wkf_d[cb * 128:(cb + 1) * 128, :])
            wvf_sb = wp.tile([128, FB, C], bf16)
            for fb in range(FB):
                nc.sync.dma_start(out=wvf_sb[:, fb, :],
                                  in_=wvf_d[fb * 128:(fb + 1) * 128, :])
            wrf_sb = wp.tile([128, CB, C], bf16)
            for cb in range(CB):
                nc.sync.dma_start(out=wrf_sb[:, cb, :],
                                  in_=wrf_d[cb * 128:(cb + 1) * 128, :])

            x2tp = ph.enter_context(tc.tile_pool(name="x2tp", bufs=3))
            statsp = ph.enter_context(tc.tile_pool(name="statsB", bufs=4))
            xnp = ph.enter_context(tc.tile_pool(name="xnpB", bufs=2))
            xnTp = ph.enter_context(tc.tile_pool(name="xnTB", bufs=1))
            mixp = ph.enter_context(tc.tile_pool(name="mixB", bufs=1))
            ffp = ph.enter_context(tc.tile_pool(name="ffp", bufs=2))
            psKV = ph.enter_context(tc.tile_pool(name="psKV", bufs=1, space="PSUM"))
            psSm = ph.enter_context(tc.tile_pool(name="psSm", bufs=2, space="PSUM"))

            NTT = TCB // 128  # 2

            for jc in range(NCHB):
                _sid, _ = nc.enter_named_scope(f"B{jc}", False)
                t0 = jc * TCB
                x2ts = []
                for tt in range(NTT):
                    x2t = x2tp.tile([128, C], f32, name="x2t", tag="x2t")
                    nc.sync.dma_start(
                        out=x2t, in_=x2_dram[t0 + tt * 128: t0 + (tt + 1) * 128, :])
                    x2ts.append(x2t)

                xnT = xnTp.tile([128, CB, TCB + 1], bf16, name="xnT2", tag="xnT2")
                if jc == 0:
                    nc.gpsimd.memset(xnT[:, :, 0], 0.0)
                else:
                    nc.gpsimd.tensor_copy(out=xnT[:, :, 0], in_=xlast2)

                for tt in range(NTT):
                    xnh = ln_apply(x2ts[tt], statsp, xnp)
                    for cb in range(CB):
                        pt = psT.tile([128, 128], f32, name="pt2", tag="pt")
                        nc.tensor.transpose(pt, xnh[:, cb * 128:(cb + 1) * 128], ident)
                        nc.scalar.copy(
                            out=xnT[:, cb, 1 + tt * 128: 1 + (tt + 1) * 128],
                            in_=pt)
                nc.gpsimd.tensor_copy(out=xlast2, in_=xnT[:, :, TCB])

                xk_t = mixp.tile([128, CB, TCB], bf16, name="xk2_t", tag="xk2")
                xr_t = mixp.tile([128, CB, TCB], bf16, name="xr2_t", tag="xr2")
                for cb in range(CB):
                    xn_v = xnT[:, cb, 1:TCB + 1]
                    xx_v = xnT[:, cb, 0:TCB]
                    dd = mixp.tile([128, TCB], bf16, name="dd2", tag="dd2", bufs=2)
                    nc.vector.tensor_sub(dd, xn_v, xx_v)
                    nc.vector.scalar_tensor_tensor(
                        out=xk_t[:, cb, :], in0=dd, scalar=par["mfk"][:, cb:cb + 1],
                        in1=xx_v, op0=OP.mult, op1=OP.add)
                    nc.vector.scalar_tensor_tensor(
                        out=xr_t[:, cb, :], in0=dd, scalar=par["mfr"][:, cb:cb + 1],
                        in1=xx_v, op0=OP.mult, op1=OP.add)

                kvps = [[psKV.tile([128, 512], f32, name=f"kv{tt}{chh}",
                                   tag=f"kv{tt}{chh}")
                         for chh in range(2)] for tt in range(NTT)]

                for fb in range(FB):
                    pskf = psSm.tile([128, TCB], f32, name="pskf", tag="sm")
                    for cb in range(CB):
                        nc.tensor.matmul(
                            pskf, wkf_sb[:, cb, fb * 128:(fb + 1) * 128],
                            xk_t[:, cb, :], start=(cb == 0), stop=(cb == CB - 1))
                    kfb = ffp.tile([128, TCB], bf16, name="kfb", tag="kf", bufs=3)
                    nc.scalar.activation(out=kfb, in_=pskf, func=AF.Relu,
                                         bias=bkf_t[:, fb:fb + 1])
                    kfs = ffp.tile([128, TCB], bf16, name="kfs", tag="kfs", bufs=3)
                    nc.vector.tensor_mul(kfs, kfb, kfb)
                    for tt in range(NTT):
                        for chh in range(2):
                            nc.tensor.matmul(
                                kvps[tt][chh],
                                kfs[:, tt * 128:(tt + 1) * 128],
                                wvf_sb[:, fb, chh * 512:(chh + 1) * 512],
                                start=(fb == 0), stop=(fb == FB - 1))

                for tt in range(NTT):
                    for chh in range(2):
                        psr2 = psSm.tile([128, 512], f32, name="psr2", tag="sm")
                        for cb in range(CB):
                            nc.tensor.matmul(
                                psr2,
                                xr_t[:, cb, tt * 128:(tt + 1) * 128],
                                wrf_sb[:, cb, chh * 512:(chh + 1) * 512],
                                start=(cb == 0), stop=(cb == CB - 1))
                        # sigmoid via exp/ln only (same ACT table set):
                        # sig = exp(-ln(1+exp(-r)))
                        if with_b2r:
                            nc.vector.tensor_add(
                                psr2, psr2,
                                b2r_t[:, chh * 512:(chh + 1) * 512])
                        er2 = ffp.tile([128, 512], f32, name="er2", tag="sr2")
                        nc.scalar.activation(out=er2, in_=psr2, func=AF.Exp,
                                             scale=-1.0)
                        ln2t = ffp.tile([128, 512], f32, name="ln2t", tag="ln2t")
                        nc.scalar.activation(out=ln2t, in_=er2, func=AF.Ln,
                                             bias=one_t)
                        sg2 = ffp.tile([128, 512], f32, name="sg2", tag="sg2")
                        nc.scalar.activation(out=sg2, in_=ln2t, func=AF.Exp,
                                             scale=-1.0)
                        tmp = ffp.tile([128, 512], f32, name="tmp", tag="tmp", bufs=1)
                        nc.vector.tensor_mul(tmp, sg2, kvps[tt][chh])
                        x3 = ffp.tile([128, 512], f32, name="x3", tag="x3")
                        nc.vector.tensor_add(
                            x3, tmp, x2ts[tt][:, chh * 512:(chh + 1) * 512])
                        nc.sync.dma_start(
                            out=y_d[t0 + tt * 128: t0 + (tt + 1) * 128,
                                    chh * 512:(chh + 1) * 512],
                            in_=x3)
                nc.leave_named_scope(f"B{jc}", _sid, False)

    nc.compile()
    return nc


def get_program(with_b2r=False):
    key = ("nc", with_b2r)
    if key not in _CACHE:
        _CACHE[key] = _build_program(with_b2r)
    return _CACHE[key]


def host_inputs(inputs):
    """Host-side precompute: transposed bf16 weights + derived per-channel vecs.
    Returns the shared (per-core-identical) part of the in_map."""
    bf = ml_dtypes.bfloat16
    f32 = np.float32

    def v(name):
        return np.asarray(inputs[name], f32).reshape(-1)

    g1, b1 = v("ln1_g"), v("ln1_b")
    g2, b2 = v("ln2_g"), v("ln2_b")
    Wk = np.asarray(inputs["Wk"], f32)
    Wv = np.asarray(inputs["Wv"], f32)
    Wr = np.asarray(inputs["Wr"], f32)
    Wkf = np.asarray(inputs["Wk_ffn"], f32)
    Wrf = np.asarray(inputs["Wr_ffn"], f32)

    def tb(a):
        return np.ascontiguousarray(a.astype(bf))

    shared = {
        # LN gain folded in: row c of W.T scaled by g[c]
        "wkT": tb(Wk.T * g1[:, None]),
        "wvT": tb(Wv.T * g1[:, None]),
        "wrT": tb(Wr.T * g1[:, None]),
        "woT": tb(np.asarray(inputs["Wo"], f32).T),
        "wkfT": tb(Wkf.T * g2[:, None]),
        "wvfT": tb(np.asarray(inputs["Wv_ffn"], f32).T),
        "wrfT": tb(Wrf.T * g2[:, None]),
        "mk": v("att_mix_k"),
        "mv": v("att_mix_v"),
        "mr": v("att_mix_r"),
        "mfk": v("ffn_mix_k"),
        "mfr": v("ffn_mix_r"),
        "lam": np.exp(-np.exp(v("time_decay"))).astype(f32),
        "eu": np.exp(v("time_first")).astype(f32),
        # LN bias folded to per-output-channel matmul bias
        "bk": (Wk @ b1).astype(f32),
        "bv": (Wv @ b1).astype(f32),
        "brn": (-(Wr @ b1)).astype(f32),
        "bkf": (Wkf @ b2).astype(f32),
    }
    b2r = (Wrf @ b2).astype(f32)
    with_b2r = bool(np.any(b2r != 0.0))
    if with_b2r:
        shared["b2r"] = b2r
    return shared, with_b2r


def _ensure_axon_hooks():
    """The agent image's antenv lacks axon_hooks; bass_utils trace=True needs
    it. Install a shim wired to the injected libaxon_pjrt.so NTFF profiler."""
    try:
        import antenv.axon_hooks  # noqa: F401
        return
    except ImportError:
        pass
    import types
    mod = types.ModuleType("antenv.axon_hooks")
    mod._hook = None

    def set_axon_ntff_profile_hook(h):
        mod._hook = h

    def get_axon_ntff_profile_hook():
        return mod._hook

    mod.set_axon_ntff_profile_hook = set_axon_ntff_profile_hook
    mod.get_axon_ntff_profile_hook = get_axon_ntff_profile_hook
    sys.modules["antenv.axon_hooks"] = mod
    import antenv
    antenv.axon_hooks = mod
    try:
        from trn_agent_boot.trn_boot import _ntff_profile_via_ctypes
        so = "/opt/axon/libaxon_pjrt.so"
        if os.path.exists(so):
            mod._hook = _ntff_profile_via_ctypes(so)
    except Exception as e:  # pragma: no cover - degrade to no tracing
        print("ntff hook unavailable:", e)


def kernel(**inputs):
    from concourse import bass_utils

    shared, with_b2r = host_inputs(inputs)
    nc = get_program(with_b2r)
    X = np.asarray(inputs["x"], np.float32)
    in_maps = [dict(shared, x=np.ascontiguousarray(X[b])) for b in range(NCORES)]

    trace = os.environ.get("KERNEL_TRACE", "0") == "1"
    if trace:
        _ensure_axon_hooks()
    res = bass_utils.run_bass_kernel_spmd(
        nc, in_maps, core_ids=list(range(NCORES)), trace=trace)
    kernel._last_exec_ns = res.exec_time_ns
    kernel._last_results = res
    out = np.stack([r["y"] for r in res.results], axis=0)
    return out


kernel._last_exec_ns = None
